# revision 26
# baseline (speedup 1.0000x reference)
"""Trainium2 Bass kernel for nn_MultiHeadAttention_824633721543.

MHA with periodic prefix mask: allowed iff (q % 256) >= (k % 256).
B=2, S=2048, D=768, H=12, Dk=64, WINDOW=256.

Sharding: 8 cores = 2 batches x 4 head-groups (3 heads each). Each core
computes q/k/v projections for its heads, the masked softmax attention, and
a partial O-projection; the host sums the 4 partials per batch and adds bo.

v2 (bf16): all matmuls in bfloat16 (1 cycle/row vs the 3-pass fp32-HIGH mode
the f32r version lowered to). Device-side layout (all transpose-free):
  - q columns tile-permuted (even 128-tiles | odd 128-tiles) so the mask is:
      even-group x k-lo  -> one shared 128x128 triu tile (0/1 multiply)
      odd-group  x k-lo  -> unmasked
      odd-group  x k-hi  -> shared triu
      even-group x k-hi  -> fully masked, never computed
  - scores as S^T [k,q]: kT slice stationary, qT moving; h0 (partitions 0:64)
    and h1 (64:128) interleaved per window so their K=64 matmuls can
    row-tile-pair in the PE array and the exp stream keeps ACT saturated
  - exp on ACT (scale=1/8 folded in), no max-subtraction (scores are small)
  - P@V with [V|1] stationary -> out^T plus denominator row, accumulated over
    windows in PSUM; normalization via K=1 broadcast matmul + DVE
  - O-projection consumes attn^T directly; h0/h1 share one [128,128]
    stationary (true head-sum via 128-deep contraction)
"""

import sys

sys.path.insert(0, "/opt/trn_rl_repo")

import numpy as np
import ml_dtypes

BF16 = ml_dtypes.bfloat16

B = 2
S = 2048
D = 768
DK = 64
WIN = 256
NW = S // WIN   # 8 windows
NHC = 3         # heads per core
DH = NHC * DK   # 192
NT = S // 128   # 16 q tiles

_CACHE = {}


def _build_program():
    import concourse.tile as tile
    from concourse import mybir, bacc
    from contextlib import ExitStack

    f32 = mybir.dt.float32
    bf16 = mybir.dt.bfloat16
    Exp = mybir.ActivationFunctionType.Exp
    mult = mybir.AluOpType.mult
    add = mybir.AluOpType.add

    nc = bacc.Bacc("TRN2", target_bir_lowering=False, debug=False)

    xT = nc.dram_tensor("xT", [D, S], bf16, kind="ExternalInput").ap()
    w1 = nc.dram_tensor("w1", [D, 256], bf16, kind="ExternalInput").ap()  # [qh0|qh1|kh0|kh1]
    w2 = nc.dram_tensor("w2", [D, 256], bf16, kind="ExternalInput").ap()  # [qh2|qh2|kh2|kh2]
    wv = nc.dram_tensor("wv", [D, 195], bf16, kind="ExternalInput").ap()
    woA = nc.dram_tensor("woA", [128, D], bf16, kind="ExternalInput").ap()  # WoT h0|h1
    woB = nc.dram_tensor("woB", [64, D], bf16, kind="ExternalInput").ap()   # WoT h2
    btA = nc.dram_tensor("btA", [128, 1], f32, kind="ExternalInput").ap()
    btB = nc.dram_tensor("btB", [128, 1], f32, kind="ExternalInput").ap()
    btC = nc.dram_tensor("btC", [128, 1], f32, kind="ExternalInput").ap()
    btD = nc.dram_tensor("btD", [128, 1], f32, kind="ExternalInput").ap()
    bvb = nc.dram_tensor("bvb", [128, 195], bf16, kind="ExternalInput").ap()
    triu = nc.dram_tensor("triu", [128, 128], bf16, kind="ExternalInput").ap()
    onesd = nc.dram_tensor("onesd", [128, 64], bf16, kind="ExternalInput").ap()
    out = nc.dram_tensor("out", [S, D], bf16, kind="ExternalOutput").ap()

    with tile.TileContext(nc) as tc, ExitStack() as ctx:
        consts = ctx.enter_context(tc.tile_pool(name="consts", bufs=1))
        qkv = ctx.enter_context(tc.tile_pool(name="qkv", bufs=1))

        xtp_cm = tc.tile_pool(name="xtp", bufs=1)
        xtp = xtp_cm.__enter__()
        xT_sb = [xtp.tile([128, S], bf16, tag=f"xt{k}", name=f"xt{k}")
                 for k in range(6)]
        w1_sb = [consts.tile([128, 256], bf16, tag=f"w1_{k}", name=f"w1s{k}")
                 for k in range(6)]
        w2_sb = [consts.tile([128, 256], bf16, tag=f"w2_{k}", name=f"w2s{k}")
                 for k in range(6)]
        wv_sb = [consts.tile([128, 195], bf16, tag=f"wv_{k}", name=f"wvs{k}")
                 for k in range(6)]
        for k in range(6):
            nc.sync.dma_start(out=xT_sb[k], in_=xT[k * 128:(k + 1) * 128, :])
            nc.sync.dma_start(out=w1_sb[k], in_=w1[k * 128:(k + 1) * 128, :])
            nc.sync.dma_start(out=w2_sb[k], in_=w2[k * 128:(k + 1) * 128, :])
            nc.sync.dma_start(out=wv_sb[k], in_=wv[k * 128:(k + 1) * 128, :])
        woA_sb = consts.tile([128, D], bf16, tag="woA")
        woB_sb = consts.tile([64, D], bf16, tag="woB")
        nc.sync.dma_start(out=woA_sb, in_=woA)
        nc.sync.dma_start(out=woB_sb, in_=woB)
        btA_sb = consts.tile([128, 1], f32, tag="btA")
        btB_sb = consts.tile([128, 1], f32, tag="btB")
        btC_sb = consts.tile([128, 1], f32, tag="btC")
        btD_sb = consts.tile([128, 1], f32, tag="btD")
        nc.sync.dma_start(out=btA_sb, in_=btA)
        nc.sync.dma_start(out=btB_sb, in_=btB)
        nc.sync.dma_start(out=btC_sb, in_=btC)
        nc.sync.dma_start(out=btD_sb, in_=btD)
        bvb_sb = consts.tile([128, 195], bf16, tag="bvb")
        nc.sync.dma_start(out=bvb_sb, in_=bvb)
        triu_sb = consts.tile([128, 128], bf16, tag="triu")
        nc.sync.dma_start(out=triu_sb, in_=triu)
        ones_row = consts.tile([128, 64], bf16, tag="ones_row")
        nc.sync.dma_start(out=ones_row, in_=onesd)

        # ---- long-lived activation tiles ----
        tileA = qkv.tile([128, S], bf16, tag="tileA")  # [qT_h0|qT_h1], q-permuted
        tileB = qkv.tile([128, S], bf16, tag="tileB")  # [kT_h0|kT_h1], natural
        tileC = qkv.tile([128, S], bf16, tag="tileC")  # qT_h2 x2, permuted
        tileD = qkv.tile([128, S], bf16, tag="tileD")  # kT_h2 x2, natural
        # v natural [s,d] per s-tile: three 65-col groups [V_h | 1]
        v_sb = [qkv.tile([128, 196], bf16, tag=f"v{i}", name=f"vsb{i}")
                for i in range(NT)]
        # attn^T: h0 at partitions 0:64, h1 at 64:128; h2 separate
        attnT01 = qkv.tile([128, S], bf16, tag="attnT01")
        attnT2 = qkv.tile([64, S], bf16, tag="attnT2")

        def mm(out, lhsT, rhs, start, stop, reuse=False, **kw):
            if not reuse:
                return nc.tensor.matmul(out, lhsT, rhs, start=start,
                                        stop=stop, **kw)
            # same stationary as the previous matmul on this row-group:
            # emit a non-self-loading InstMatmult (ldweights=False) so the
            # PE streams without a redundant weight reload
            te = nc.tensor
            ifmap_ap = te.lower_ap(rhs.opt({0}), opt=False)
            weights_ap = te.lower_ap(lhsT.opt({0}), opt=False,
                                     for_matmul_weights=True)
            out_ap = te.lower_ap(out)

            def rup(s):
                return 32 if s <= 32 else (64 if s <= 64 else 128)

            return te.add_instruction(mybir.InstMatmult(
                name=te.bass.get_next_instruction_name(),
                replication_resolution=0,
                replication_shift_amnt=0,
                replication_num_rows=0,
                start_tensor_calc=start,
                stop_tensor_calc=stop,
                ins=[ifmap_ap, weights_ap],
                outs=[out_ap],
                perf_mode=None,
                is_transpose=None,
                ifmap_quant_offset=None,
                weights_quant_offset=None,
                bass_skip_group_check=False,
                tile_position=(lhsT.base_partition(), out.base_partition()),
                tile_size=(rup(rhs.partition_size()),
                           rup(out.partition_size())),
                ldweights=False,
            ))

        def permuted_copy(dst, rows, ps, n, bias):
            """psum 512-span n -> dst cols with even/odd tile permutation."""
            pr3 = ps[0:rows, :].rearrange("p (c two k) -> p c two k", two=2, k=128)
            dr = dst[0:rows, :]
            nc.vector.tensor_scalar_add(
                out=dr[:, 256 * n:256 * n + 256].rearrange("p (c k) -> p c k", k=128),
                in0=pr3[:, :, 0, :], scalar1=bias[0:rows, :])
            nc.vector.tensor_scalar_add(
                out=dr[:, 1024 + 256 * n:1024 + 256 * n + 256].rearrange(
                    "p (c k) -> p c k", k=128),
                in0=pr3[:, :, 1, :], scalar1=bias[0:rows, :])

        # ---- stage A ----
        with tc.tile_pool(name="psA", bufs=2, space="PSUM") as psA:
            for n in range(4):
                xn = [xT_sb[k][:, 512 * n:512 * (n + 1)]
                      for k in range(6)]
                psa = psA.tile([128, 512], f32, tag="psA")
                for k in range(6):
                    nc.tensor.matmul(psa, w1_sb[k][:, 0:128], xn[k],
                                     start=(k == 0), stop=(k == 5))
                permuted_copy(tileA, 128, psa, n, btA_sb)
                psb = psA.tile([128, 512], f32, tag="psA")
                for k in range(6):
                    nc.tensor.matmul(psb, w1_sb[k][:, 128:256], xn[k],
                                     start=(k == 0), stop=(k == 5))
                nc.vector.tensor_scalar_add(
                    out=tileB[:, 512 * n:512 * (n + 1)], in0=psb, scalar1=btB_sb)
                psq = psA.tile([128, 512], f32, tag="psq")
                psk = psA.tile([128, 512], f32, tag="psq")
                for k in range(6):
                    nc.tensor.matmul(psq, w2_sb[k][:, 0:128], xn[k],
                                     start=(k == 0), stop=(k == 5))
                    nc.tensor.matmul(psk, w2_sb[k][:, 128:256], xn[k],
                                     start=(k == 0), stop=(k == 5))
                permuted_copy(tileC, 128, psq, n, btC_sb)
                nc.vector.tensor_scalar_add(
                    out=tileD[:, 512 * n:512 * (n + 1)], in0=psk, scalar1=btD_sb)

            def v_proj(st, psv):
                for k in range(6):
                    nc.tensor.matmul(
                        psv, xT_sb[k][:, 128 * st:128 * (st + 1)],
                        wv_sb[k], start=(k == 0), stop=(k == 5))
                vt = v_sb[st]
                # ones columns baked in: wv cols {64,129,194}=0, bvb there=1
                nc.vector.tensor_tensor(
                    out=vt[:, 0:195], in0=psv, in1=bvb_sb, op=add)

            # even s-tiles now (feed grp0's P@V); odd tiles deferred into
            # stage B's grp0 loop to fill PE gaps there
            for st in range(0, NT, 2):
                psv = psA.tile([128, 195], f32, tag="psv")
                v_proj(st, psv)

        # ---- stage B + interleaved stage C ----
        triu_b = triu_sb.unsqueeze(1).broadcast_to([128, 8, 128])

        with tc.tile_pool(name="pt", bufs=10) as pt_pool, \
             tc.tile_pool(name="sc", bufs=2, space="PSUM") as sc_pool, \
             tc.tile_pool(name="po", bufs=2, space="PSUM") as out_pool, \
             tc.tile_pool(name="posb", bufs=3) as posb_pool, \
             tc.tile_pool(name="nrm", bufs=3) as nrm_pool, \
             tc.tile_pool(name="ost", bufs=3) as ost_pool:

            def scores_exp(kblk, qcols, mask, nm):
                sc = sc_pool.tile([128, 1024], f32, tag="sc")
                for sub in range(2):
                    mm(sc[:, 512 * sub:512 * (sub + 1)], kblk,
                       qcols[:, 512 * sub:512 * (sub + 1)],
                       start=True, stop=True, reuse=(sub == 1))
                pt = pt_pool.tile([128, 1024], bf16, tag="pt", name=nm)
                nc.scalar.activation(out=pt, in_=sc, func=Exp, scale=0.125)
                if mask:
                    p3 = pt.rearrange("p (c k) -> p c k", k=128)
                    nc.vector.tensor_mul(out=p3, in0=p3, in1=triu_b)
                return pt

            def scores_exp_pair(kblks, qcolss, masks, nm):
                """two K=64 score blocks on disjoint row-groups (partitions
                0:64 / 64:128) with the matmuls interleaved so adjacent MMs
                can run concurrently as row tiles in the PE array."""
                scs = [sc_pool.tile([128, 1024], f32, tag="sc",
                                    name=f"{nm}s{h}") for h in range(2)]
                for sub in range(2):
                    for h in range(2):
                        mm(scs[h][:, 512 * sub:512 * (sub + 1)], kblks[h],
                           qcolss[h][:, 512 * sub:512 * (sub + 1)],
                           start=True, stop=True, reuse=(sub == 1))
                pts = []
                for h in range(2):
                    pt = pt_pool.tile([128, 1024], bf16, tag="pt",
                                      name=f"{nm}p{h}")
                    nc.scalar.activation(out=pt, in_=scs[h], func=Exp,
                                         scale=0.125)
                    pts.append(pt)
                for h in range(2):
                    if masks[h]:
                        p3 = pts[h].rearrange("p (c k) -> p c k", k=128)
                        nc.vector.tensor_mul(out=p3, in0=p3, in1=triu_b)
                return pts

            def pv_mm(po, vtile, h, pt, first, last):
                vsl = vtile[:, 65 * h:65 * h + 65]  # [V_h | 1]
                for sub in range(2):
                    mm(po[0:65, 512 * sub:512 * (sub + 1)], vsl,
                       pt[:, 512 * sub:512 * (sub + 1)],
                       start=first, stop=last, reuse=(sub == 1))

            def drain(po, nm):
                """po [65,1024] psum -> SBUF bf16, freeing the psum bank."""
                po_sb = posb_pool.tile([65, 1024], bf16, tag="posb", name=nm)
                nc.vector.tensor_copy(out=po_sb, in_=po[0:65, :])
                return po_sb

            def norm_emit(po_sb, dstv, grp):
                """normalize drained po into an attnT slice (PE+DVE, lazy)."""
                rec_ps = sc_pool.tile([128, 1024], f32, tag="sc")
                for sub in range(2):
                    mm(rec_ps[0:64, 512 * sub:512 * (sub + 1)],
                       ones_row[64:65, :],
                       po_sb[64:65, 512 * sub:512 * (sub + 1)],
                       start=True, stop=True, reuse=(sub == 1))
                rec_sb = nrm_pool.tile([64, 1024], f32, tag="rec")
                nc.vector.reciprocal_approx_fast(
                    out=rec_sb, in_=rec_ps[0:64, :])
                nc.vector.tensor_tensor(
                    out=dstv[:, 1024 * grp:1024 * (grp + 1)],
                    in0=po_sb[0:64, :], in1=rec_sb, op=mult)

            def c_tile(p, copy_eng, pool=None, ptag="sc"):
                """one O-projection tile: 4 MMs + psum->sbuf copy + DMA."""
                psof = (pool or sc_pool).tile([128, 1024], f32, tag=ptag,
                                              name=f"pso{p}")
                pso = psof[:, 0:D]
                s01 = attnT01[:, 128 * p:128 * (p + 1)]
                s2 = attnT2[:, 128 * p:128 * (p + 1)]
                for i, (n0, n1) in enumerate(((0, 512), (512, 768))):
                    mm(pso[:, n0:n1], s01, woA_sb[:, n0:n1],
                       start=True, stop=False, reuse=(i == 1))
                for i, (n0, n1) in enumerate(((0, 512), (512, 768))):
                    mm(pso[:, n0:n1], s2, woB_sb[:, n0:n1],
                       start=False, stop=True, reuse=(i == 1))
                ot = ost_pool.tile([128, D], bf16, tag="ot")
                copy_eng(out=ot, in_=pso)
                t = 2 * p if p < 8 else 2 * (p - 8) + 1
                nc.sync.dma_start(out=out[128 * t:128 * (t + 1), :], in_=ot)

            # --- h0 & h1 row-tile-paired; PV lags scores by one window;
            #     odd-tile V projections fill grp0's PE gaps ---
            pending_norm = []
            for grp in range(2):  # 0=even q-tiles, 1=odd
                po01 = [out_pool.tile([128, 1024], f32, tag="po",
                                      name=f"po{grp}_{i}")
                        for i in range(2)]
                qvs = [tileA[64 * h:64 * h + 64,
                             1024 * grp:1024 * (grp + 1)] for h in range(2)]
                pend = [None, None]  # per head: list of (pt, vtile, first)
                for w in range(NW):
                    klos = [tileB[64 * h:64 * h + 64, WIN * w:WIN * w + 128]
                            for h in range(2)]
                    if grp == 0:
                        pts = scores_exp_pair(klos, qvs, (True, True), f"g0w{w}")
                        blocks = [[(pts[h], v_sb[2 * w])] for h in range(2)]
                    else:
                        ptl = scores_exp_pair(klos, qvs, (False, False), f"g1w{w}l")
                        khis = [tileB[64 * h:64 * h + 64,
                                      WIN * w + 128:WIN * w + 256]
                                for h in range(2)]
                        pth = scores_exp_pair(khis, qvs, (True, True), f"g1w{w}u")
                        blocks = [[(ptl[h], v_sb[2 * w]),
                                   (pth[h], v_sb[2 * w + 1])]
                                  for h in range(2)]
                    for h in range(2):
                        if pend[h] is not None:
                            for pt_, vt_, fi_ in pend[h]:
                                pv_mm(po01[h], vt_, h, pt_, fi_, False)
                        pend[h] = [(b[0], b[1], (w == 0 and i == 0))
                                   for i, b in enumerate(blocks[h])]
                    if grp == 0:
                        # odd V tile 2w+1 (psum borrowed from the sc pool)
                        psvf = sc_pool.tile([128, 1024], f32, tag="sc",
                                            name=f"psv{w}")
                        v_proj(2 * w + 1, psvf[:, 0:195])
                    if w == 0 and pending_norm:
                        for args in pending_norm:
                            norm_emit(*args)
                        pending_norm = []
                for h in range(2):
                    n = len(pend[h])
                    for i, (pt_, vt_, fi_) in enumerate(pend[h]):
                        pv_mm(po01[h], vt_, h, pt_, fi_, i == n - 1)
                    po_sb = drain(po01[h], f"posb{grp}_{h}")
                    pending_norm.append(
                        (po_sb, attnT01[64 * h:64 * h + 64, :], grp))


            # --- h2 (q/k duplicated on both partition halves so blocks
            #     row-tile-pair); stage C even q-tiles interleaved in grp1 ---
            for grp in range(2):
                po = out_pool.tile([128, 1024], f32, tag="po",
                                   name=f"po2_{grp}")
                qvh = [tileC[64 * j:64 * j + 64,
                             1024 * grp:1024 * (grp + 1)] for j in range(2)]
                pend = None
                for it in range(4 if grp == 0 else NW):
                    if grp == 0:
                        # windows 2it (rows 0:64) and 2it+1 (rows 64:128)
                        w0, w1 = 2 * it, 2 * it + 1
                        kbs = [tileD[0:64, WIN * w0:WIN * w0 + 128],
                               tileD[64:128, WIN * w1:WIN * w1 + 128]]
                        pts = scores_exp_pair(kbs, qvh, (True, True),
                                              f"h2g0i{it}")
                        blocks = [(pts[0], v_sb[2 * w0]),
                                  (pts[1], v_sb[2 * w1])]
                    else:
                        w = it
                        kbs = [tileD[0:64, WIN * w:WIN * w + 128],
                               tileD[64:128, WIN * w + 128:WIN * w + 256]]
                        pts = scores_exp_pair(kbs, qvh, (False, True),
                                              f"h2g1i{it}")
                        blocks = [(pts[0], v_sb[2 * w]),
                                  (pts[1], v_sb[2 * w + 1])]
                    if pend is not None:
                        for pt_, vt_, fi_ in pend:
                            pv_mm(po, vt_, 2, pt_, fi_, False)
                    pend = [(b[0], b[1], (it == 0 and i == 0))
                            for i, b in enumerate(blocks)]
                    if it == 0 and pending_norm:
                        for args in pending_norm:
                            norm_emit(*args)
                        pending_norm = []
                    if grp == 1 and it >= 1:
                        c_tile(it - 1, nc.vector.tensor_copy)
                n = len(pend)
                for i, (pt_, vt_, fi_) in enumerate(pend):
                    pv_mm(po, vt_, 2, pt_, fi_, i == n - 1)
                po_sb = drain(po, f"posb2_{grp}")
                pending_norm.append((po_sb, attnT2, grp))

            c_tile(7, nc.vector.tensor_copy)
            for args in pending_norm:
                norm_emit(*args)
            pending_norm = []

            # --- stage C tail: odd q-tiles, two psum pools in flight ---
            engs = [nc.scalar.copy, nc.vector.tensor_copy]
            for p in range(8, NT):
                if p % 2:
                    c_tile(p, engs[p % 2], pool=out_pool, ptag="po")
                else:
                    c_tile(p, engs[p % 2])

        xtp_cm.__exit__(None, None, None)

    nc.compile()
    return nc


def _prep_core_inputs(inputs, c):
    x = inputs["x"]
    Wq, bq = inputs["Wq"], inputs["bq"]
    Wk, bk = inputs["Wk"], inputs["bk"]
    Wv, bv = inputs["Wv"], inputs["bv"]
    Wo = inputs["Wo"]
    b = c // 4
    r0 = (c % 4) * DH  # first feature row of this core's 192-row head block

    xT = np.ascontiguousarray(np.asarray(x[b]).T.astype(BF16))
    W1 = np.ascontiguousarray(np.concatenate(
        [Wq[r0:r0 + 128].T, Wk[r0:r0 + 128].T], axis=1).astype(BF16))
    q2 = Wq[r0 + 128:r0 + 192].T
    k2 = Wk[r0 + 128:r0 + 192].T
    W2 = np.ascontiguousarray(np.concatenate([q2, q2, k2, k2], axis=1).astype(BF16))
    Wvp = np.zeros((D, 195), np.float32)
    for h in range(3):
        Wvp[:, 65 * h:65 * h + 64] = Wv[r0 + 64 * h:r0 + 64 * h + 64].T
    Wvp = np.ascontiguousarray(Wvp.astype(BF16))
    bvbr = np.zeros((195,), np.float32)
    for h in range(3):
        bvbr[65 * h:65 * h + 64] = bv[r0 + 64 * h:r0 + 64 * h + 64]
        bvbr[65 * h + 64] = 1.0
    woA = np.ascontiguousarray(Wo[:, r0:r0 + 128].T.astype(BF16))
    woB = np.ascontiguousarray(Wo[:, r0 + 128:r0 + 192].T.astype(BF16))

    return dict(
        xT=xT, w1=W1, w2=W2, wv=Wvp, woA=woA, woB=woB,
        btA=np.ascontiguousarray(bq[r0:r0 + 128].reshape(128, 1).astype(np.float32)),
        btB=np.ascontiguousarray(bk[r0:r0 + 128].reshape(128, 1).astype(np.float32)),
        btC=np.ascontiguousarray(np.tile(
            bq[r0 + 128:r0 + 192], 2).reshape(128, 1).astype(np.float32)),
        btD=np.ascontiguousarray(np.tile(
            bk[r0 + 128:r0 + 192], 2).reshape(128, 1).astype(np.float32)),
        bvb=np.ascontiguousarray(np.tile(
            bvbr.reshape(1, 195), (128, 1)).astype(BF16)),
        triu=np.ascontiguousarray(np.triu(np.ones((128, 128), np.float32))).astype(BF16),
        onesd=np.ones((128, 64), BF16),
    )


def _install_ntff_hook():
    """Register antenv.axon_hooks with a ctypes NTFF profile hook so
    run_bass_kernel_spmd(trace=True) can capture device-side exec time."""
    import types, ctypes, contextlib

    try:
        import antenv.axon_hooks  # noqa: F401
        return
    except ImportError:
        pass
    so_path = "/opt/axon/libaxon_pjrt.so"
    lib = ctypes.CDLL(so_path)
    if not hasattr(lib, "axon_start_nrt_profile"):
        return
    lib.axon_start_nrt_profile.argtypes = [
        ctypes.POINTER(ctypes.c_int64), ctypes.c_size_t]
    lib.axon_start_nrt_profile.restype = ctypes.c_int64
    lib.axon_stop_nrt_profile.argtypes = [ctypes.c_char_p]
    lib.axon_stop_nrt_profile.restype = ctypes.c_int64

    @contextlib.contextmanager
    def _hook(output_dir, device_ids):
        import jax
        jax.devices()
        if device_ids:
            ids = (ctypes.c_int64 * len(device_ids))(*device_ids)
            rc = lib.axon_start_nrt_profile(ids, len(device_ids))
        else:
            rc = lib.axon_start_nrt_profile(None, 0)
        if rc != 0:
            raise RuntimeError(f"axon_start_nrt_profile rc={rc}")
        try:
            yield
        finally:
            n = lib.axon_stop_nrt_profile(str(output_dir).encode())
            print(f"profile: {n} file(s) written to {output_dir}")

    mod = types.ModuleType("antenv.axon_hooks")
    mod.get_axon_ntff_profile_hook = lambda: _hook
    mod.set_axon_ntff_profile_hook = lambda h: None
    sys.modules["antenv.axon_hooks"] = mod
    import antenv
    antenv.axon_hooks = mod


def kernel(**inputs):
    import os
    from concourse import bass_utils

    if "nc" not in _CACHE:
        _CACHE["nc"] = _build_program()
    nc = _CACHE["nc"]

    trace = bool(os.environ.get("MHA_TRACE"))
    kwargs = {}
    if trace:
        _install_ntff_hook()
        kwargs = dict(trace=True, tmpdir="/tmp/mha_trace")
        os.makedirs("/tmp/mha_trace", exist_ok=True)

    in_maps = [_prep_core_inputs(inputs, c) for c in range(8)]
    res = bass_utils.run_bass_kernel_spmd(
        nc, in_maps, core_ids=list(range(8)), **kwargs)
    _CACHE["last_results"] = res
    if trace and res.exec_time_ns is not None:
        print(f"HW exec time: {res.exec_time_ns} ns")
    out = np.zeros((B, S, D), np.float32)
    for c in range(8):
        out[c // 4] += res.results[c]["out"].astype(np.float32)
    out += np.asarray(inputs["bo"], np.float32).reshape(1, 1, D)
    return out


# revision 27
# speedup vs baseline: 1.2999x; 1.2999x over previous
"""Trainium2 Bass kernel for nn_MultiHeadAttention_824633721543.

MHA with periodic prefix mask: allowed iff (q % 256) >= (k % 256).
B=2, S=2048, D=768, H=12, Dk=64, WINDOW=256.

Sharding: 8 cores = 2 batches x 4 head-groups (3 heads each). Each core
computes q/k/v projections for its heads, the masked softmax attention, and
a partial O-projection; the host sums the 4 partials per batch and adds bo.

v2 (bf16): all matmuls in bfloat16 (1 cycle/row vs the 3-pass fp32-HIGH mode
the f32r version lowered to). Device-side layout (all transpose-free):
  - q columns tile-permuted (even 128-tiles | odd 128-tiles) so the mask is:
      even-group x k-lo  -> one shared 128x128 triu tile (0/1 multiply)
      odd-group  x k-lo  -> unmasked
      odd-group  x k-hi  -> shared triu
      even-group x k-hi  -> fully masked, never computed
  - scores as S^T [k,q]: kT slice stationary, qT moving; h0 (partitions 0:64)
    and h1 (64:128) interleaved per window so their K=64 matmuls can
    row-tile-pair in the PE array and the exp stream keeps ACT saturated
  - exp on ACT (scale=1/8 folded in), no max-subtraction (scores are small)
  - P@V with [V|1] stationary -> out^T plus denominator row, accumulated over
    windows in PSUM; normalization via K=1 broadcast matmul + DVE
  - O-projection consumes attn^T directly; h0/h1 share one [128,128]
    stationary (true head-sum via 128-deep contraction)
"""

import sys

sys.path.insert(0, "/opt/trn_rl_repo")

import numpy as np
import ml_dtypes

BF16 = ml_dtypes.bfloat16

B = 2
S = 2048
D = 768
DK = 64
WIN = 256
NW = S // WIN   # 8 windows
NHC = 3         # heads per core
DH = NHC * DK   # 192
NT = S // 128   # 16 q tiles

_CACHE = {}


def _build_program():
    import concourse.tile as tile
    from concourse import mybir, bacc
    from contextlib import ExitStack

    f32 = mybir.dt.float32
    bf16 = mybir.dt.bfloat16
    Exp = mybir.ActivationFunctionType.Exp
    mult = mybir.AluOpType.mult
    add = mybir.AluOpType.add

    nc = bacc.Bacc("TRN2", target_bir_lowering=False, debug=False)

    xT = nc.dram_tensor("xT", [D, S], bf16, kind="ExternalInput").ap()
    w1 = nc.dram_tensor("w1", [D, 256], bf16, kind="ExternalInput").ap()  # [qh0|qh1|kh0|kh1]
    w2 = nc.dram_tensor("w2", [D, 256], bf16, kind="ExternalInput").ap()  # [qh2|qh2|kh2|kh2]
    wv = nc.dram_tensor("wv", [D, 195], bf16, kind="ExternalInput").ap()
    woA = nc.dram_tensor("woA", [128, D], bf16, kind="ExternalInput").ap()  # WoT h0|h1
    woB = nc.dram_tensor("woB", [64, D], bf16, kind="ExternalInput").ap()   # WoT h2
    btA = nc.dram_tensor("btA", [128, 1], f32, kind="ExternalInput").ap()
    btB = nc.dram_tensor("btB", [128, 1], f32, kind="ExternalInput").ap()
    btC = nc.dram_tensor("btC", [128, 1], f32, kind="ExternalInput").ap()
    btD = nc.dram_tensor("btD", [128, 1], f32, kind="ExternalInput").ap()
    bvb = nc.dram_tensor("bvb", [128, 195], bf16, kind="ExternalInput").ap()
    triu = nc.dram_tensor("triu", [128, 128], bf16, kind="ExternalInput").ap()
    onesd = nc.dram_tensor("onesd", [128, 64], bf16, kind="ExternalInput").ap()
    out = nc.dram_tensor("out", [S, D], bf16, kind="ExternalOutput").ap()

    with tile.TileContext(nc) as tc, ExitStack() as ctx:
        consts = ctx.enter_context(tc.tile_pool(name="consts", bufs=1))
        qkv = ctx.enter_context(tc.tile_pool(name="qkv", bufs=1))

        xtp_cm = tc.tile_pool(name="xtp", bufs=1)
        xtp = xtp_cm.__enter__()
        xT_sb = [xtp.tile([128, S], bf16, tag=f"xt{k}", name=f"xt{k}")
                 for k in range(6)]
        w1_sb = [consts.tile([128, 256], bf16, tag=f"w1_{k}", name=f"w1s{k}")
                 for k in range(6)]
        w2_sb = [consts.tile([128, 256], bf16, tag=f"w2_{k}", name=f"w2s{k}")
                 for k in range(6)]
        wv_sb = [consts.tile([128, 195], bf16, tag=f"wv_{k}", name=f"wvs{k}")
                 for k in range(6)]
        for k in range(6):
            nc.sync.dma_start(out=xT_sb[k], in_=xT[k * 128:(k + 1) * 128, :])
            nc.sync.dma_start(out=w1_sb[k], in_=w1[k * 128:(k + 1) * 128, :])
            nc.sync.dma_start(out=w2_sb[k], in_=w2[k * 128:(k + 1) * 128, :])
            nc.sync.dma_start(out=wv_sb[k], in_=wv[k * 128:(k + 1) * 128, :])
        woA_sb = consts.tile([128, D], bf16, tag="woA")
        woB_sb = consts.tile([64, D], bf16, tag="woB")
        nc.sync.dma_start(out=woA_sb, in_=woA)
        nc.sync.dma_start(out=woB_sb, in_=woB)
        btA_sb = consts.tile([128, 1], f32, tag="btA")
        btB_sb = consts.tile([128, 1], f32, tag="btB")
        btC_sb = consts.tile([128, 1], f32, tag="btC")
        btD_sb = consts.tile([128, 1], f32, tag="btD")
        nc.sync.dma_start(out=btA_sb, in_=btA)
        nc.sync.dma_start(out=btB_sb, in_=btB)
        nc.sync.dma_start(out=btC_sb, in_=btC)
        nc.sync.dma_start(out=btD_sb, in_=btD)
        bvb_sb = consts.tile([128, 195], bf16, tag="bvb")
        nc.sync.dma_start(out=bvb_sb, in_=bvb)
        triu_sb = consts.tile([128, 128], bf16, tag="triu")
        nc.sync.dma_start(out=triu_sb, in_=triu)
        ones_row = consts.tile([128, 64], bf16, tag="ones_row")
        nc.sync.dma_start(out=ones_row, in_=onesd)

        # ---- long-lived activation tiles ----
        tileA = qkv.tile([128, S], bf16, tag="tileA")  # [qT_h0|qT_h1], q-permuted
        tileB = qkv.tile([128, S], bf16, tag="tileB")  # [kT_h0|kT_h1], natural
        tileC = qkv.tile([128, S], bf16, tag="tileC")  # qT_h2 x2, permuted
        tileD = qkv.tile([128, S], bf16, tag="tileD")  # kT_h2 x2, natural
        # v natural [s,d] per s-tile: three 65-col groups [V_h | 1]
        v_sb = [qkv.tile([128, 196], bf16, tag=f"v{i}", name=f"vsb{i}")
                for i in range(NT)]
        # attn^T: h0 at partitions 0:64, h1 at 64:128; h2 separate
        attnT01 = qkv.tile([128, S], bf16, tag="attnT01")
        attnT2 = qkv.tile([64, S], bf16, tag="attnT2")

        def mm(out, lhsT, rhs, start, stop, reuse=False, **kw):
            if not reuse:
                return nc.tensor.matmul(out, lhsT, rhs, start=start,
                                        stop=stop, **kw)
            # same stationary as the previous matmul on this row-group:
            # emit a non-self-loading InstMatmult (ldweights=False) so the
            # PE streams without a redundant weight reload
            te = nc.tensor
            ifmap_ap = te.lower_ap(rhs.opt({0}), opt=False)
            weights_ap = te.lower_ap(lhsT.opt({0}), opt=False,
                                     for_matmul_weights=True)
            out_ap = te.lower_ap(out)

            def rup(s):
                return 32 if s <= 32 else (64 if s <= 64 else 128)

            return te.add_instruction(mybir.InstMatmult(
                name=te.bass.get_next_instruction_name(),
                replication_resolution=0,
                replication_shift_amnt=0,
                replication_num_rows=0,
                start_tensor_calc=start,
                stop_tensor_calc=stop,
                ins=[ifmap_ap, weights_ap],
                outs=[out_ap],
                perf_mode=None,
                is_transpose=None,
                ifmap_quant_offset=None,
                weights_quant_offset=None,
                bass_skip_group_check=False,
                tile_position=(lhsT.base_partition(), out.base_partition()),
                tile_size=(rup(rhs.partition_size()),
                           rup(out.partition_size())),
                ldweights=False,
            ))

        def permuted_copy(dst, rows, ps, n, bias):
            """psum 512-span n -> dst cols with even/odd tile permutation."""
            pr3 = ps[0:rows, :].rearrange("p (c two k) -> p c two k", two=2, k=128)
            dr = dst[0:rows, :]
            nc.vector.tensor_scalar_add(
                out=dr[:, 256 * n:256 * n + 256].rearrange("p (c k) -> p c k", k=128),
                in0=pr3[:, :, 0, :], scalar1=bias[0:rows, :])
            nc.vector.tensor_scalar_add(
                out=dr[:, 1024 + 256 * n:1024 + 256 * n + 256].rearrange(
                    "p (c k) -> p c k", k=128),
                in0=pr3[:, :, 1, :], scalar1=bias[0:rows, :])

        # ---- stage A ----
        with tc.tile_pool(name="psA", bufs=2, space="PSUM") as psA:
            for n in range(4):
                xn = [xT_sb[k][:, 512 * n:512 * (n + 1)]
                      for k in range(6)]
                psa = psA.tile([128, 512], f32, tag="psA")
                for k in range(6):
                    nc.tensor.matmul(psa, w1_sb[k][:, 0:128], xn[k],
                                     start=(k == 0), stop=(k == 5))
                permuted_copy(tileA, 128, psa, n, btA_sb)
                psb = psA.tile([128, 512], f32, tag="psA")
                for k in range(6):
                    nc.tensor.matmul(psb, w1_sb[k][:, 128:256], xn[k],
                                     start=(k == 0), stop=(k == 5))
                nc.vector.tensor_scalar_add(
                    out=tileB[:, 512 * n:512 * (n + 1)], in0=psb, scalar1=btB_sb)
                psq = psA.tile([128, 512], f32, tag="psq")
                psk = psA.tile([128, 512], f32, tag="psq")
                for k in range(6):
                    nc.tensor.matmul(psq, w2_sb[k][:, 0:128], xn[k],
                                     start=(k == 0), stop=(k == 5))
                    nc.tensor.matmul(psk, w2_sb[k][:, 128:256], xn[k],
                                     start=(k == 0), stop=(k == 5))
                permuted_copy(tileC, 128, psq, n, btC_sb)
                nc.vector.tensor_scalar_add(
                    out=tileD[:, 512 * n:512 * (n + 1)], in0=psk, scalar1=btD_sb)

            def v_proj(st, psv):
                for k in range(6):
                    nc.tensor.matmul(
                        psv, xT_sb[k][:, 128 * st:128 * (st + 1)],
                        wv_sb[k], start=(k == 0), stop=(k == 5))
                vt = v_sb[st]
                # ones columns baked in: wv cols {64,129,194}=0, bvb there=1
                nc.vector.tensor_tensor(
                    out=vt[:, 0:195], in0=psv, in1=bvb_sb, op=add)

            # even s-tiles now (feed grp0's P@V); odd tiles deferred into
            # stage B's grp0 loop to fill PE gaps there
            for st in range(0, NT, 2):
                psv = psA.tile([128, 195], f32, tag="psv")
                v_proj(st, psv)

        # ---- stage B + interleaved stage C ----
        triu_b = triu_sb.unsqueeze(1).broadcast_to([128, 8, 128])

        with tc.tile_pool(name="pt", bufs=8) as pt_pool, \
             tc.tile_pool(name="sc", bufs=2, space="PSUM") as sc_pool, \
             tc.tile_pool(name="po", bufs=2, space="PSUM") as out_pool, \
             tc.tile_pool(name="posb", bufs=2) as posb_pool, \
             tc.tile_pool(name="nrm", bufs=2) as nrm_pool, \
             tc.tile_pool(name="ost", bufs=3) as ost_pool:

            def scores_exp(kblk, qcols, mask, nm):
                sc = sc_pool.tile([128, 1024], f32, tag="sc")
                for sub in range(2):
                    mm(sc[:, 512 * sub:512 * (sub + 1)], kblk,
                       qcols[:, 512 * sub:512 * (sub + 1)],
                       start=True, stop=True, reuse=(sub == 1))
                pt = pt_pool.tile([128, 1024], bf16, tag="pt", name=nm)
                nc.scalar.activation(out=pt, in_=sc, func=Exp, scale=0.125)
                if mask:
                    p3 = pt.rearrange("p (c k) -> p c k", k=128)
                    nc.vector.tensor_mul(out=p3, in0=p3, in1=triu_b)
                return pt

            def scores_exp_pair(kblks, qcolss, masks, nm):
                """two K=64 score blocks on disjoint row-groups (partitions
                0:64 / 64:128) with the matmuls interleaved so adjacent MMs
                can run concurrently as row tiles in the PE array."""
                scs = [sc_pool.tile([128, 1024], f32, tag="sc",
                                    name=f"{nm}s{h}") for h in range(2)]
                for sub in range(2):
                    for h in range(2):
                        mm(scs[h][:, 512 * sub:512 * (sub + 1)], kblks[h],
                           qcolss[h][:, 512 * sub:512 * (sub + 1)],
                           start=True, stop=True, reuse=(sub == 1))
                pts = []
                for h in range(2):
                    pt = pt_pool.tile([128, 1024], bf16, tag="pt",
                                      name=f"{nm}p{h}")
                    nc.scalar.activation(out=pt, in_=scs[h], func=Exp,
                                         scale=0.125)
                    pts.append(pt)
                for h in range(2):
                    if masks[h]:
                        p3 = pts[h].rearrange("p (c k) -> p c k", k=128)
                        nc.vector.tensor_mul(out=p3, in0=p3, in1=triu_b)
                return pts

            def pv_mm(po, vtile, h, pt, first, last):
                vsl = vtile[:, 65 * h:65 * h + 65]  # [V_h | 1]
                for sub in range(2):
                    mm(po[0:65, 512 * sub:512 * (sub + 1)], vsl,
                       pt[:, 512 * sub:512 * (sub + 1)],
                       start=first, stop=last, reuse=(sub == 1))

            def drain(po, nm):
                """po [65,1024] psum -> SBUF bf16, freeing the psum bank."""
                po_sb = posb_pool.tile([65, 1024], bf16, tag="posb", name=nm)
                nc.vector.tensor_copy(out=po_sb, in_=po[0:65, :])
                return po_sb

            def norm_emit(po_sb, dstv, grp):
                """normalize drained po into an attnT slice (PE+DVE, lazy)."""
                rec_ps = sc_pool.tile([128, 1024], f32, tag="sc")
                for sub in range(2):
                    mm(rec_ps[0:64, 512 * sub:512 * (sub + 1)],
                       ones_row[64:65, :],
                       po_sb[64:65, 512 * sub:512 * (sub + 1)],
                       start=True, stop=True, reuse=(sub == 1))
                rec_sb = nrm_pool.tile([64, 1024], f32, tag="rec")
                nc.vector.reciprocal_approx_fast(
                    out=rec_sb, in_=rec_ps[0:64, :])
                nc.vector.tensor_tensor(
                    out=dstv[:, 1024 * grp:1024 * (grp + 1)],
                    in0=po_sb[0:64, :], in1=rec_sb, op=mult)

            def c_tile(p, copy_eng, pool=None, ptag="sc"):
                """one O-projection tile: 4 MMs + psum->sbuf copy + DMA."""
                psof = (pool or sc_pool).tile([128, 1024], f32, tag=ptag,
                                              name=f"pso{p}")
                pso = psof[:, 0:D]
                s01 = attnT01[:, 128 * p:128 * (p + 1)]
                s2 = attnT2[:, 128 * p:128 * (p + 1)]
                for i, (n0, n1) in enumerate(((0, 512), (512, 768))):
                    mm(pso[:, n0:n1], s01, woA_sb[:, n0:n1],
                       start=True, stop=False, reuse=(i == 1))
                for i, (n0, n1) in enumerate(((0, 512), (512, 768))):
                    mm(pso[:, n0:n1], s2, woB_sb[:, n0:n1],
                       start=False, stop=True, reuse=(i == 1))
                ot = ost_pool.tile([128, D], bf16, tag="ot")
                copy_eng(out=ot, in_=pso)
                t = 2 * p if p < 8 else 2 * (p - 8) + 1
                nc.sync.dma_start(out=out[128 * t:128 * (t + 1), :], in_=ot)

            # --- h0 & h1 row-tile-paired; PV lags scores by one window;
            #     odd-tile V projections fill grp0's PE gaps ---
            pending_norm = []
            for grp in range(2):  # 0=even q-tiles, 1=odd
                po01 = [out_pool.tile([128, 1024], f32, tag="po",
                                      name=f"po{grp}_{i}")
                        for i in range(2)]
                qvs = [tileA[64 * h:64 * h + 64,
                             1024 * grp:1024 * (grp + 1)] for h in range(2)]
                pend = [None, None]  # per head: list of (pt, vtile, first)
                for w in range(NW):
                    klos = [tileB[64 * h:64 * h + 64, WIN * w:WIN * w + 128]
                            for h in range(2)]
                    if grp == 0:
                        pts = scores_exp_pair(klos, qvs, (True, True), f"g0w{w}")
                        blocks = [[(pts[h], v_sb[2 * w])] for h in range(2)]
                    else:
                        ptl = scores_exp_pair(klos, qvs, (False, False), f"g1w{w}l")
                        khis = [tileB[64 * h:64 * h + 64,
                                      WIN * w + 128:WIN * w + 256]
                                for h in range(2)]
                        pth = scores_exp_pair(khis, qvs, (True, True), f"g1w{w}u")
                        blocks = [[(ptl[h], v_sb[2 * w]),
                                   (pth[h], v_sb[2 * w + 1])]
                                  for h in range(2)]
                    for h in range(2):
                        if pend[h] is not None:
                            for pt_, vt_, fi_ in pend[h]:
                                pv_mm(po01[h], vt_, h, pt_, fi_, False)
                        pend[h] = [(b[0], b[1], (w == 0 and i == 0))
                                   for i, b in enumerate(blocks[h])]
                    if grp == 0:
                        # odd V tile 2w+1 (psum borrowed from the sc pool)
                        psvf = sc_pool.tile([128, 1024], f32, tag="sc",
                                            name=f"psv{w}")
                        v_proj(2 * w + 1, psvf[:, 0:195])
                    if w == 1 and pending_norm:
                        for args in pending_norm:
                            norm_emit(*args)
                        pending_norm = []
                for h in range(2):
                    n = len(pend[h])
                    for i, (pt_, vt_, fi_) in enumerate(pend[h]):
                        pv_mm(po01[h], vt_, h, pt_, fi_, i == n - 1)
                    po_sb = drain(po01[h], f"posb{grp}_{h}")
                    pending_norm.append(
                        (po_sb, attnT01[64 * h:64 * h + 64, :], grp))


            # --- h2 (q/k duplicated on both partition halves so blocks
            #     row-tile-pair); stage C even q-tiles interleaved in grp1 ---
            for grp in range(2):
                po = out_pool.tile([128, 1024], f32, tag="po",
                                   name=f"po2_{grp}")
                qvh = [tileC[64 * j:64 * j + 64,
                             1024 * grp:1024 * (grp + 1)] for j in range(2)]
                pend = None
                for it in range(4 if grp == 0 else NW):
                    if grp == 0:
                        # windows 2it (rows 0:64) and 2it+1 (rows 64:128)
                        w0, w1 = 2 * it, 2 * it + 1
                        kbs = [tileD[0:64, WIN * w0:WIN * w0 + 128],
                               tileD[64:128, WIN * w1:WIN * w1 + 128]]
                        pts = scores_exp_pair(kbs, qvh, (True, True),
                                              f"h2g0i{it}")
                        blocks = [(pts[0], v_sb[2 * w0]),
                                  (pts[1], v_sb[2 * w1])]
                    else:
                        w = it
                        kbs = [tileD[0:64, WIN * w:WIN * w + 128],
                               tileD[64:128, WIN * w + 128:WIN * w + 256]]
                        pts = scores_exp_pair(kbs, qvh, (False, True),
                                              f"h2g1i{it}")
                        blocks = [(pts[0], v_sb[2 * w]),
                                  (pts[1], v_sb[2 * w + 1])]
                    if pend is not None:
                        for pt_, vt_, fi_ in pend:
                            pv_mm(po, vt_, 2, pt_, fi_, False)
                    pend = [(b[0], b[1], (it == 0 and i == 0))
                            for i, b in enumerate(blocks)]
                    if it == 1 and pending_norm:
                        for args in pending_norm:
                            norm_emit(*args)
                        pending_norm = []
                    if grp == 1 and it >= 1:
                        c_tile(it - 1, nc.vector.tensor_copy)
                n = len(pend)
                for i, (pt_, vt_, fi_) in enumerate(pend):
                    pv_mm(po, vt_, 2, pt_, fi_, i == n - 1)
                po_sb = drain(po, f"posb2_{grp}")
                pending_norm.append((po_sb, attnT2, grp))

            c_tile(7, nc.vector.tensor_copy)
            for args in pending_norm:
                norm_emit(*args)
            pending_norm = []

            # --- stage C tail: odd q-tiles, two psum pools in flight ---
            engs = [nc.scalar.copy, nc.vector.tensor_copy]
            for p in range(8, NT):
                if p % 2:
                    c_tile(p, engs[p % 2], pool=out_pool, ptag="po")
                else:
                    c_tile(p, engs[p % 2])

        xtp_cm.__exit__(None, None, None)

    nc.compile()
    return nc


def _prep_core_inputs(inputs, c):
    x = inputs["x"]
    Wq, bq = inputs["Wq"], inputs["bq"]
    Wk, bk = inputs["Wk"], inputs["bk"]
    Wv, bv = inputs["Wv"], inputs["bv"]
    Wo = inputs["Wo"]
    b = c // 4
    r0 = (c % 4) * DH  # first feature row of this core's 192-row head block

    xT = np.ascontiguousarray(np.asarray(x[b]).T.astype(BF16))
    W1 = np.ascontiguousarray(np.concatenate(
        [Wq[r0:r0 + 128].T, Wk[r0:r0 + 128].T], axis=1).astype(BF16))
    q2 = Wq[r0 + 128:r0 + 192].T
    k2 = Wk[r0 + 128:r0 + 192].T
    W2 = np.ascontiguousarray(np.concatenate([q2, q2, k2, k2], axis=1).astype(BF16))
    Wvp = np.zeros((D, 195), np.float32)
    for h in range(3):
        Wvp[:, 65 * h:65 * h + 64] = Wv[r0 + 64 * h:r0 + 64 * h + 64].T
    Wvp = np.ascontiguousarray(Wvp.astype(BF16))
    bvbr = np.zeros((195,), np.float32)
    for h in range(3):
        bvbr[65 * h:65 * h + 64] = bv[r0 + 64 * h:r0 + 64 * h + 64]
        bvbr[65 * h + 64] = 1.0
    woA = np.ascontiguousarray(Wo[:, r0:r0 + 128].T.astype(BF16))
    woB = np.ascontiguousarray(Wo[:, r0 + 128:r0 + 192].T.astype(BF16))

    return dict(
        xT=xT, w1=W1, w2=W2, wv=Wvp, woA=woA, woB=woB,
        btA=np.ascontiguousarray(bq[r0:r0 + 128].reshape(128, 1).astype(np.float32)),
        btB=np.ascontiguousarray(bk[r0:r0 + 128].reshape(128, 1).astype(np.float32)),
        btC=np.ascontiguousarray(np.tile(
            bq[r0 + 128:r0 + 192], 2).reshape(128, 1).astype(np.float32)),
        btD=np.ascontiguousarray(np.tile(
            bk[r0 + 128:r0 + 192], 2).reshape(128, 1).astype(np.float32)),
        bvb=np.ascontiguousarray(np.tile(
            bvbr.reshape(1, 195), (128, 1)).astype(BF16)),
        triu=np.ascontiguousarray(np.triu(np.ones((128, 128), np.float32))).astype(BF16),
        onesd=np.ones((128, 64), BF16),
    )


def _install_ntff_hook():
    """Register antenv.axon_hooks with a ctypes NTFF profile hook so
    run_bass_kernel_spmd(trace=True) can capture device-side exec time."""
    import types, ctypes, contextlib

    try:
        import antenv.axon_hooks  # noqa: F401
        return
    except ImportError:
        pass
    so_path = "/opt/axon/libaxon_pjrt.so"
    lib = ctypes.CDLL(so_path)
    if not hasattr(lib, "axon_start_nrt_profile"):
        return
    lib.axon_start_nrt_profile.argtypes = [
        ctypes.POINTER(ctypes.c_int64), ctypes.c_size_t]
    lib.axon_start_nrt_profile.restype = ctypes.c_int64
    lib.axon_stop_nrt_profile.argtypes = [ctypes.c_char_p]
    lib.axon_stop_nrt_profile.restype = ctypes.c_int64

    @contextlib.contextmanager
    def _hook(output_dir, device_ids):
        import jax
        jax.devices()
        if device_ids:
            ids = (ctypes.c_int64 * len(device_ids))(*device_ids)
            rc = lib.axon_start_nrt_profile(ids, len(device_ids))
        else:
            rc = lib.axon_start_nrt_profile(None, 0)
        if rc != 0:
            raise RuntimeError(f"axon_start_nrt_profile rc={rc}")
        try:
            yield
        finally:
            n = lib.axon_stop_nrt_profile(str(output_dir).encode())
            print(f"profile: {n} file(s) written to {output_dir}")

    mod = types.ModuleType("antenv.axon_hooks")
    mod.get_axon_ntff_profile_hook = lambda: _hook
    mod.set_axon_ntff_profile_hook = lambda h: None
    sys.modules["antenv.axon_hooks"] = mod
    import antenv
    antenv.axon_hooks = mod


def kernel(**inputs):
    import os
    from concourse import bass_utils

    if "nc" not in _CACHE:
        _CACHE["nc"] = _build_program()
    nc = _CACHE["nc"]

    trace = bool(os.environ.get("MHA_TRACE"))
    kwargs = {}
    if trace:
        _install_ntff_hook()
        kwargs = dict(trace=True, tmpdir="/tmp/mha_trace")
        os.makedirs("/tmp/mha_trace", exist_ok=True)

    in_maps = [_prep_core_inputs(inputs, c) for c in range(8)]
    res = bass_utils.run_bass_kernel_spmd(
        nc, in_maps, core_ids=list(range(8)), **kwargs)
    _CACHE["last_results"] = res
    if trace and res.exec_time_ns is not None:
        print(f"HW exec time: {res.exec_time_ns} ns")
    out = np.zeros((B, S, D), np.float32)
    for c in range(8):
        out[c // 4] += res.results[c]["out"].astype(np.float32)
    out += np.asarray(inputs["bo"], np.float32).reshape(1, 1, D)
    return out


# revision 28
# speedup vs baseline: 1.3245x; 1.0189x over previous
"""Trainium2 Bass kernel for nn_MultiHeadAttention_824633721543.

MHA with periodic prefix mask: allowed iff (q % 256) >= (k % 256).
B=2, S=2048, D=768, H=12, Dk=64, WINDOW=256.

Sharding: 8 cores = 2 batches x 4 head-groups (3 heads each). Each core
computes q/k/v projections for its heads, the masked softmax attention, and
a partial O-projection; the host sums the 4 partials per batch and adds bo.

v2 (bf16): all matmuls in bfloat16 (1 cycle/row vs the 3-pass fp32-HIGH mode
the f32r version lowered to). Device-side layout (all transpose-free):
  - q columns tile-permuted (even 128-tiles | odd 128-tiles) so the mask is:
      even-group x k-lo  -> one shared 128x128 triu tile (0/1 multiply)
      odd-group  x k-lo  -> unmasked
      odd-group  x k-hi  -> shared triu
      even-group x k-hi  -> fully masked, never computed
  - scores as S^T [k,q]: kT slice stationary, qT moving; h0 (partitions 0:64)
    and h1 (64:128) interleaved per window so their K=64 matmuls can
    row-tile-pair in the PE array and the exp stream keeps ACT saturated
  - exp on ACT (scale=1/8 folded in), no max-subtraction (scores are small)
  - P@V with [V|1] stationary -> out^T plus denominator row, accumulated over
    windows in PSUM; normalization via K=1 broadcast matmul + DVE
  - O-projection consumes attn^T directly; h0/h1 share one [128,128]
    stationary (true head-sum via 128-deep contraction)
"""

import sys

sys.path.insert(0, "/opt/trn_rl_repo")

import numpy as np
import ml_dtypes

BF16 = ml_dtypes.bfloat16

B = 2
S = 2048
D = 768
DK = 64
WIN = 256
NW = S // WIN   # 8 windows
NHC = 3         # heads per core
DH = NHC * DK   # 192
NT = S // 128   # 16 q tiles

_CACHE = {}


def _build_program():
    import concourse.tile as tile
    from concourse import mybir, bacc
    from contextlib import ExitStack

    f32 = mybir.dt.float32
    bf16 = mybir.dt.bfloat16
    Exp = mybir.ActivationFunctionType.Exp
    mult = mybir.AluOpType.mult
    add = mybir.AluOpType.add

    nc = bacc.Bacc("TRN2", target_bir_lowering=False, debug=False)

    xT = nc.dram_tensor("xT", [D, S], bf16, kind="ExternalInput").ap()
    w1 = nc.dram_tensor("w1", [D, 256], bf16, kind="ExternalInput").ap()  # [qh0|qh1|kh0|kh1]
    w2 = nc.dram_tensor("w2", [D, 256], bf16, kind="ExternalInput").ap()  # [qh2|qh2|kh2|kh2]
    wv = nc.dram_tensor("wv", [D, 195], bf16, kind="ExternalInput").ap()
    woA = nc.dram_tensor("woA", [128, D], bf16, kind="ExternalInput").ap()  # WoT h0|h1
    woB = nc.dram_tensor("woB", [64, D], bf16, kind="ExternalInput").ap()   # WoT h2
    btA = nc.dram_tensor("btA", [128, 1], f32, kind="ExternalInput").ap()
    btB = nc.dram_tensor("btB", [128, 1], f32, kind="ExternalInput").ap()
    btC = nc.dram_tensor("btC", [128, 1], f32, kind="ExternalInput").ap()
    btD = nc.dram_tensor("btD", [128, 1], f32, kind="ExternalInput").ap()
    bvb = nc.dram_tensor("bvb", [128, 195], bf16, kind="ExternalInput").ap()
    triu = nc.dram_tensor("triu", [128, 128], bf16, kind="ExternalInput").ap()
    onesd = nc.dram_tensor("onesd", [128, 64], bf16, kind="ExternalInput").ap()
    out = nc.dram_tensor("out", [S, D], bf16, kind="ExternalOutput").ap()

    with tile.TileContext(nc) as tc, ExitStack() as ctx:
        consts = ctx.enter_context(tc.tile_pool(name="consts", bufs=1))
        qkv = ctx.enter_context(tc.tile_pool(name="qkv", bufs=1))

        xtp_cm = tc.tile_pool(name="xtp", bufs=1)
        xtp = xtp_cm.__enter__()
        xT_sb = [xtp.tile([128, S], bf16, tag=f"xt{k}", name=f"xt{k}")
                 for k in range(6)]
        w1_sb = [consts.tile([128, 256], bf16, tag=f"w1_{k}", name=f"w1s{k}")
                 for k in range(6)]
        w2_sb = [consts.tile([128, 256], bf16, tag=f"w2_{k}", name=f"w2s{k}")
                 for k in range(6)]
        wv_sb = [consts.tile([128, 195], bf16, tag=f"wv_{k}", name=f"wvs{k}")
                 for k in range(6)]
        for k in range(6):
            nc.sync.dma_start(out=xT_sb[k], in_=xT[k * 128:(k + 1) * 128, :])
            nc.sync.dma_start(out=w1_sb[k], in_=w1[k * 128:(k + 1) * 128, :])
            nc.sync.dma_start(out=w2_sb[k], in_=w2[k * 128:(k + 1) * 128, :])
            nc.sync.dma_start(out=wv_sb[k], in_=wv[k * 128:(k + 1) * 128, :])
        woA_sb = consts.tile([128, D], bf16, tag="woA")
        woB_sb = consts.tile([64, D], bf16, tag="woB")
        nc.sync.dma_start(out=woA_sb, in_=woA)
        nc.sync.dma_start(out=woB_sb, in_=woB)
        btA_sb = consts.tile([128, 1], f32, tag="btA")
        btB_sb = consts.tile([128, 1], f32, tag="btB")
        btC_sb = consts.tile([128, 1], f32, tag="btC")
        btD_sb = consts.tile([128, 1], f32, tag="btD")
        nc.sync.dma_start(out=btA_sb, in_=btA)
        nc.sync.dma_start(out=btB_sb, in_=btB)
        nc.sync.dma_start(out=btC_sb, in_=btC)
        nc.sync.dma_start(out=btD_sb, in_=btD)
        bvb_sb = consts.tile([128, 195], bf16, tag="bvb")
        nc.sync.dma_start(out=bvb_sb, in_=bvb)
        triu_sb = consts.tile([128, 128], bf16, tag="triu")
        nc.sync.dma_start(out=triu_sb, in_=triu)
        ones_row = consts.tile([128, 64], bf16, tag="ones_row")
        nc.sync.dma_start(out=ones_row, in_=onesd)

        # ---- long-lived activation tiles ----
        tileA = qkv.tile([128, S], bf16, tag="tileA")  # [qT_h0|qT_h1], q-permuted
        tileB = qkv.tile([128, S], bf16, tag="tileB")  # [kT_h0|kT_h1], natural
        tileC = qkv.tile([128, S], bf16, tag="tileC")  # qT_h2 x2, permuted
        tileD = qkv.tile([128, S], bf16, tag="tileD")  # kT_h2 x2, natural
        # v natural [s,d] per s-tile: three 65-col groups [V_h | 1]
        v_sb = [qkv.tile([128, 196], bf16, tag=f"v{i}", name=f"vsb{i}")
                for i in range(NT)]
        # attn^T: h0 at partitions 0:64, h1 at 64:128; h2 separate
        attnT01 = qkv.tile([128, S], bf16, tag="attnT01")
        attnT2 = qkv.tile([64, S], bf16, tag="attnT2")

        def mm(out, lhsT, rhs, start, stop, reuse=False, **kw):
            return nc.tensor.matmul(out, lhsT, rhs, start=start, stop=stop,
                                    **kw)

        def permuted_copy(dst, rows, ps, n, bias):
            """psum 512-span n -> dst cols with even/odd tile permutation."""
            pr3 = ps[0:rows, :].rearrange("p (c two k) -> p c two k", two=2, k=128)
            dr = dst[0:rows, :]
            nc.vector.tensor_scalar_add(
                out=dr[:, 256 * n:256 * n + 256].rearrange("p (c k) -> p c k", k=128),
                in0=pr3[:, :, 0, :], scalar1=bias[0:rows, :])
            nc.vector.tensor_scalar_add(
                out=dr[:, 1024 + 256 * n:1024 + 256 * n + 256].rearrange(
                    "p (c k) -> p c k", k=128),
                in0=pr3[:, :, 1, :], scalar1=bias[0:rows, :])

        # ---- stage A ----
        with tc.tile_pool(name="psA", bufs=2, space="PSUM") as psA:
            for n in range(4):
                xn = [xT_sb[k][:, 512 * n:512 * (n + 1)]
                      for k in range(6)]
                psa = psA.tile([128, 512], f32, tag="psA")
                for k in range(6):
                    nc.tensor.matmul(psa, w1_sb[k][:, 0:128], xn[k],
                                     start=(k == 0), stop=(k == 5))
                permuted_copy(tileA, 128, psa, n, btA_sb)
                psb = psA.tile([128, 512], f32, tag="psA")
                for k in range(6):
                    nc.tensor.matmul(psb, w1_sb[k][:, 128:256], xn[k],
                                     start=(k == 0), stop=(k == 5))
                nc.vector.tensor_scalar_add(
                    out=tileB[:, 512 * n:512 * (n + 1)], in0=psb, scalar1=btB_sb)
                psq = psA.tile([128, 512], f32, tag="psq")
                psk = psA.tile([128, 512], f32, tag="psq")
                for k in range(6):
                    nc.tensor.matmul(psq, w2_sb[k][:, 0:128], xn[k],
                                     start=(k == 0), stop=(k == 5))
                    nc.tensor.matmul(psk, w2_sb[k][:, 128:256], xn[k],
                                     start=(k == 0), stop=(k == 5))
                permuted_copy(tileC, 128, psq, n, btC_sb)
                nc.vector.tensor_scalar_add(
                    out=tileD[:, 512 * n:512 * (n + 1)], in0=psk, scalar1=btD_sb)

            def v_proj(st, psv):
                for k in range(6):
                    nc.tensor.matmul(
                        psv, xT_sb[k][:, 128 * st:128 * (st + 1)],
                        wv_sb[k], start=(k == 0), stop=(k == 5))
                vt = v_sb[st]
                # ones columns baked in: wv cols {64,129,194}=0, bvb there=1
                nc.vector.tensor_tensor(
                    out=vt[:, 0:195], in0=psv, in1=bvb_sb, op=add)

            # even s-tiles now (feed grp0's P@V); odd tiles deferred into
            # stage B's grp0 loop to fill PE gaps there
            for st in range(0, NT, 2):
                psv = psA.tile([128, 195], f32, tag="psv")
                v_proj(st, psv)

        # ---- stage B + interleaved stage C ----
        triu_b = triu_sb.unsqueeze(1).broadcast_to([128, 8, 128])

        with tc.tile_pool(name="pt", bufs=8) as pt_pool, \
             tc.tile_pool(name="sc", bufs=2, space="PSUM") as sc_pool, \
             tc.tile_pool(name="po", bufs=2, space="PSUM") as out_pool, \
             tc.tile_pool(name="posb", bufs=2) as posb_pool, \
             tc.tile_pool(name="nrm", bufs=2) as nrm_pool, \
             tc.tile_pool(name="ost", bufs=3) as ost_pool:

            def scores_exp(kblk, qcols, mask, nm):
                sc = sc_pool.tile([128, 1024], f32, tag="sc")
                for sub in range(2):
                    mm(sc[:, 512 * sub:512 * (sub + 1)], kblk,
                       qcols[:, 512 * sub:512 * (sub + 1)],
                       start=True, stop=True, reuse=(sub == 1))
                pt = pt_pool.tile([128, 1024], bf16, tag="pt", name=nm)
                nc.scalar.activation(out=pt, in_=sc, func=Exp, scale=0.125)
                if mask:
                    p3 = pt.rearrange("p (c k) -> p c k", k=128)
                    nc.vector.tensor_mul(out=p3, in0=p3, in1=triu_b)
                return pt

            def scores_exp_pair(kblks, qcolss, masks, nm):
                """two K=64 score blocks on disjoint row-groups (partitions
                0:64 / 64:128) with the matmuls interleaved so adjacent MMs
                can run concurrently as row tiles in the PE array."""
                scs = [sc_pool.tile([128, 1024], f32, tag="sc",
                                    name=f"{nm}s{h}") for h in range(2)]
                for sub in range(2):
                    for h in range(2):
                        mm(scs[h][:, 512 * sub:512 * (sub + 1)], kblks[h],
                           qcolss[h][:, 512 * sub:512 * (sub + 1)],
                           start=True, stop=True, reuse=(sub == 1))
                pts = []
                for h in range(2):
                    pt = pt_pool.tile([128, 1024], bf16, tag="pt",
                                      name=f"{nm}p{h}")
                    nc.scalar.activation(out=pt, in_=scs[h], func=Exp,
                                         scale=0.125)
                    pts.append(pt)
                for h in range(2):
                    if masks[h]:
                        p3 = pts[h].rearrange("p (c k) -> p c k", k=128)
                        nc.vector.tensor_mul(out=p3, in0=p3, in1=triu_b)
                return pts

            def pv_mm(po, vtile, h, pt, first, last):
                vsl = vtile[:, 65 * h:65 * h + 65]  # [V_h | 1]
                for sub in range(2):
                    mm(po[0:65, 512 * sub:512 * (sub + 1)], vsl,
                       pt[:, 512 * sub:512 * (sub + 1)],
                       start=first, stop=last, reuse=(sub == 1))

            def drain(po, nm):
                """po [65,1024] psum -> SBUF bf16, freeing the psum bank."""
                po_sb = posb_pool.tile([65, 1024], bf16, tag="posb", name=nm)
                nc.vector.tensor_copy(out=po_sb, in_=po[0:65, :])
                return po_sb

            def norm_emit(po_sb, dstv, grp):
                """normalize drained po into an attnT slice (PE+DVE, lazy)."""
                rec_ps = sc_pool.tile([128, 1024], f32, tag="sc")
                for sub in range(2):
                    mm(rec_ps[0:64, 512 * sub:512 * (sub + 1)],
                       ones_row[64:65, :],
                       po_sb[64:65, 512 * sub:512 * (sub + 1)],
                       start=True, stop=True, reuse=(sub == 1))
                rec_sb = nrm_pool.tile([64, 1024], f32, tag="rec")
                nc.vector.reciprocal_approx_fast(
                    out=rec_sb, in_=rec_ps[0:64, :])
                nc.vector.tensor_tensor(
                    out=dstv[:, 1024 * grp:1024 * (grp + 1)],
                    in0=po_sb[0:64, :], in1=rec_sb, op=mult)

            def c_tile(p, copy_eng, pool=None, ptag="sc"):
                """one O-projection tile: 4 MMs + psum->sbuf copy + DMA."""
                psof = (pool or sc_pool).tile([128, 1024], f32, tag=ptag,
                                              name=f"pso{p}")
                pso = psof[:, 0:D]
                s01 = attnT01[:, 128 * p:128 * (p + 1)]
                s2 = attnT2[:, 128 * p:128 * (p + 1)]
                for i, (n0, n1) in enumerate(((0, 512), (512, 768))):
                    mm(pso[:, n0:n1], s01, woA_sb[:, n0:n1],
                       start=True, stop=False, reuse=(i == 1))
                for i, (n0, n1) in enumerate(((0, 512), (512, 768))):
                    mm(pso[:, n0:n1], s2, woB_sb[:, n0:n1],
                       start=False, stop=True, reuse=(i == 1))
                ot = ost_pool.tile([128, D], bf16, tag="ot")
                copy_eng(out=ot, in_=pso)
                t = 2 * p if p < 8 else 2 * (p - 8) + 1
                nc.sync.dma_start(out=out[128 * t:128 * (t + 1), :], in_=ot)

            # --- h0 & h1 row-tile-paired; PV lags scores by one window;
            #     odd-tile V projections fill grp0's PE gaps ---
            pending_norm = []
            for grp in range(2):  # 0=even q-tiles, 1=odd
                po01 = [out_pool.tile([128, 1024], f32, tag="po",
                                      name=f"po{grp}_{i}")
                        for i in range(2)]
                qvs = [tileA[64 * h:64 * h + 64,
                             1024 * grp:1024 * (grp + 1)] for h in range(2)]
                pend = [None, None]  # per head: list of (pt, vtile, first)
                for w in range(NW):
                    klos = [tileB[64 * h:64 * h + 64, WIN * w:WIN * w + 128]
                            for h in range(2)]
                    if grp == 0:
                        pts = scores_exp_pair(klos, qvs, (True, True), f"g0w{w}")
                        blocks = [[(pts[h], v_sb[2 * w])] for h in range(2)]
                    else:
                        ptl = scores_exp_pair(klos, qvs, (False, False), f"g1w{w}l")
                        khis = [tileB[64 * h:64 * h + 64,
                                      WIN * w + 128:WIN * w + 256]
                                for h in range(2)]
                        pth = scores_exp_pair(khis, qvs, (True, True), f"g1w{w}u")
                        blocks = [[(ptl[h], v_sb[2 * w]),
                                   (pth[h], v_sb[2 * w + 1])]
                                  for h in range(2)]
                    for h in range(2):
                        if pend[h] is not None:
                            for pt_, vt_, fi_ in pend[h]:
                                pv_mm(po01[h], vt_, h, pt_, fi_, False)
                        pend[h] = [(b[0], b[1], (w == 0 and i == 0))
                                   for i, b in enumerate(blocks[h])]
                    if grp == 0:
                        # odd V tile 2w+1 (psum borrowed from the sc pool)
                        psvf = sc_pool.tile([128, 1024], f32, tag="sc",
                                            name=f"psv{w}")
                        v_proj(2 * w + 1, psvf[:, 0:195])
                    if w == 1 and pending_norm:
                        for args in pending_norm:
                            norm_emit(*args)
                        pending_norm = []
                for h in range(2):
                    n = len(pend[h])
                    for i, (pt_, vt_, fi_) in enumerate(pend[h]):
                        pv_mm(po01[h], vt_, h, pt_, fi_, i == n - 1)
                    po_sb = drain(po01[h], f"posb{grp}_{h}")
                    pending_norm.append(
                        (po_sb, attnT01[64 * h:64 * h + 64, :], grp))


            # --- h2 (q/k duplicated on both partition halves so blocks
            #     row-tile-pair); stage C even q-tiles interleaved in grp1 ---
            for grp in range(2):
                po = out_pool.tile([128, 1024], f32, tag="po",
                                   name=f"po2_{grp}")
                qvh = [tileC[64 * j:64 * j + 64,
                             1024 * grp:1024 * (grp + 1)] for j in range(2)]
                pend = None
                for it in range(4 if grp == 0 else NW):
                    if grp == 0:
                        # windows 2it (rows 0:64) and 2it+1 (rows 64:128)
                        w0, w1 = 2 * it, 2 * it + 1
                        kbs = [tileD[0:64, WIN * w0:WIN * w0 + 128],
                               tileD[64:128, WIN * w1:WIN * w1 + 128]]
                        pts = scores_exp_pair(kbs, qvh, (True, True),
                                              f"h2g0i{it}")
                        blocks = [(pts[0], v_sb[2 * w0]),
                                  (pts[1], v_sb[2 * w1])]
                    else:
                        w = it
                        kbs = [tileD[0:64, WIN * w:WIN * w + 128],
                               tileD[64:128, WIN * w + 128:WIN * w + 256]]
                        pts = scores_exp_pair(kbs, qvh, (False, True),
                                              f"h2g1i{it}")
                        blocks = [(pts[0], v_sb[2 * w]),
                                  (pts[1], v_sb[2 * w + 1])]
                    if pend is not None:
                        for pt_, vt_, fi_ in pend:
                            pv_mm(po, vt_, 2, pt_, fi_, False)
                    pend = [(b[0], b[1], (it == 0 and i == 0))
                            for i, b in enumerate(blocks)]
                    if it == 1 and pending_norm:
                        for args in pending_norm:
                            norm_emit(*args)
                        pending_norm = []
                    if grp == 1 and it >= 1:
                        c_tile(it - 1, nc.vector.tensor_copy)
                n = len(pend)
                for i, (pt_, vt_, fi_) in enumerate(pend):
                    pv_mm(po, vt_, 2, pt_, fi_, i == n - 1)
                po_sb = drain(po, f"posb2_{grp}")
                pending_norm.append((po_sb, attnT2, grp))

            c_tile(7, nc.vector.tensor_copy)
            for args in pending_norm:
                norm_emit(*args)
            pending_norm = []

            # --- stage C tail: odd q-tiles, two psum pools in flight ---
            engs = [nc.scalar.copy, nc.vector.tensor_copy]
            for p in range(8, NT):
                if p % 2:
                    c_tile(p, engs[p % 2], pool=out_pool, ptag="po")
                else:
                    c_tile(p, engs[p % 2])

        xtp_cm.__exit__(None, None, None)

    nc.compile()
    return nc


def _prep_core_inputs(inputs, c):
    x = inputs["x"]
    Wq, bq = inputs["Wq"], inputs["bq"]
    Wk, bk = inputs["Wk"], inputs["bk"]
    Wv, bv = inputs["Wv"], inputs["bv"]
    Wo = inputs["Wo"]
    b = c // 4
    r0 = (c % 4) * DH  # first feature row of this core's 192-row head block

    xT = np.ascontiguousarray(np.asarray(x[b]).T.astype(BF16))
    W1 = np.ascontiguousarray(np.concatenate(
        [Wq[r0:r0 + 128].T, Wk[r0:r0 + 128].T], axis=1).astype(BF16))
    q2 = Wq[r0 + 128:r0 + 192].T
    k2 = Wk[r0 + 128:r0 + 192].T
    W2 = np.ascontiguousarray(np.concatenate([q2, q2, k2, k2], axis=1).astype(BF16))
    Wvp = np.zeros((D, 195), np.float32)
    for h in range(3):
        Wvp[:, 65 * h:65 * h + 64] = Wv[r0 + 64 * h:r0 + 64 * h + 64].T
    Wvp = np.ascontiguousarray(Wvp.astype(BF16))
    bvbr = np.zeros((195,), np.float32)
    for h in range(3):
        bvbr[65 * h:65 * h + 64] = bv[r0 + 64 * h:r0 + 64 * h + 64]
        bvbr[65 * h + 64] = 1.0
    woA = np.ascontiguousarray(Wo[:, r0:r0 + 128].T.astype(BF16))
    woB = np.ascontiguousarray(Wo[:, r0 + 128:r0 + 192].T.astype(BF16))

    return dict(
        xT=xT, w1=W1, w2=W2, wv=Wvp, woA=woA, woB=woB,
        btA=np.ascontiguousarray(bq[r0:r0 + 128].reshape(128, 1).astype(np.float32)),
        btB=np.ascontiguousarray(bk[r0:r0 + 128].reshape(128, 1).astype(np.float32)),
        btC=np.ascontiguousarray(np.tile(
            bq[r0 + 128:r0 + 192], 2).reshape(128, 1).astype(np.float32)),
        btD=np.ascontiguousarray(np.tile(
            bk[r0 + 128:r0 + 192], 2).reshape(128, 1).astype(np.float32)),
        bvb=np.ascontiguousarray(np.tile(
            bvbr.reshape(1, 195), (128, 1)).astype(BF16)),
        triu=np.ascontiguousarray(np.triu(np.ones((128, 128), np.float32))).astype(BF16),
        onesd=np.ones((128, 64), BF16),
    )


def _install_ntff_hook():
    """Register antenv.axon_hooks with a ctypes NTFF profile hook so
    run_bass_kernel_spmd(trace=True) can capture device-side exec time."""
    import types, ctypes, contextlib

    try:
        import antenv.axon_hooks  # noqa: F401
        return
    except ImportError:
        pass
    so_path = "/opt/axon/libaxon_pjrt.so"
    lib = ctypes.CDLL(so_path)
    if not hasattr(lib, "axon_start_nrt_profile"):
        return
    lib.axon_start_nrt_profile.argtypes = [
        ctypes.POINTER(ctypes.c_int64), ctypes.c_size_t]
    lib.axon_start_nrt_profile.restype = ctypes.c_int64
    lib.axon_stop_nrt_profile.argtypes = [ctypes.c_char_p]
    lib.axon_stop_nrt_profile.restype = ctypes.c_int64

    @contextlib.contextmanager
    def _hook(output_dir, device_ids):
        import jax
        jax.devices()
        if device_ids:
            ids = (ctypes.c_int64 * len(device_ids))(*device_ids)
            rc = lib.axon_start_nrt_profile(ids, len(device_ids))
        else:
            rc = lib.axon_start_nrt_profile(None, 0)
        if rc != 0:
            raise RuntimeError(f"axon_start_nrt_profile rc={rc}")
        try:
            yield
        finally:
            n = lib.axon_stop_nrt_profile(str(output_dir).encode())
            print(f"profile: {n} file(s) written to {output_dir}")

    mod = types.ModuleType("antenv.axon_hooks")
    mod.get_axon_ntff_profile_hook = lambda: _hook
    mod.set_axon_ntff_profile_hook = lambda h: None
    sys.modules["antenv.axon_hooks"] = mod
    import antenv
    antenv.axon_hooks = mod


def kernel(**inputs):
    import os
    from concourse import bass_utils

    if "nc" not in _CACHE:
        _CACHE["nc"] = _build_program()
    nc = _CACHE["nc"]

    trace = bool(os.environ.get("MHA_TRACE"))
    kwargs = {}
    if trace:
        _install_ntff_hook()
        kwargs = dict(trace=True, tmpdir="/tmp/mha_trace")
        os.makedirs("/tmp/mha_trace", exist_ok=True)

    in_maps = [_prep_core_inputs(inputs, c) for c in range(8)]
    res = bass_utils.run_bass_kernel_spmd(
        nc, in_maps, core_ids=list(range(8)), **kwargs)
    _CACHE["last_results"] = res
    if trace and res.exec_time_ns is not None:
        print(f"HW exec time: {res.exec_time_ns} ns")
    out = np.zeros((B, S, D), np.float32)
    for c in range(8):
        out[c // 4] += res.results[c]["out"].astype(np.float32)
    out += np.asarray(inputs["bo"], np.float32).reshape(1, 1, D)
    return out


# revision 29
# speedup vs baseline: 1.3341x; 1.0073x over previous
"""Trainium2 Bass kernel for nn_MultiHeadAttention_824633721543.

MHA with periodic prefix mask: allowed iff (q % 256) >= (k % 256).
B=2, S=2048, D=768, H=12, Dk=64, WINDOW=256.

Sharding: 8 cores = 2 batches x 4 head-groups (3 heads each). Each core
computes q/k/v projections for its heads, the masked softmax attention, and
a partial O-projection; the host sums the 4 partials per batch and adds bo.

v2 (bf16, 274us f32r baseline -> ~185us): all matmuls in bfloat16 (1
cycle/row vs the 3-pass fp32-HIGH mode float32r lowers to on TRN2).
Device-side layout (all transpose-free):
  - q columns tile-permuted (even 128-tiles | odd 128-tiles) so the mask is:
      even-group x k-lo  -> one shared 128x128 triu tile (0/1 multiply, DVE)
      odd-group  x k-lo  -> unmasked
      odd-group  x k-hi  -> shared triu
      even-group x k-hi  -> fully masked, never computed
  - scores as S^T [k,q]: kT slice stationary, qT moving; h0 (partitions 0:64)
    and h1 (64:128) emitted interleaved so their K=64 matmuls row-tile-pair
    (concurrent sub-arrays); h2's q/k are written duplicated on both
    partition halves (widened projection matmuls, no extra cost) so h2's
    blocks pair the same way across windows / lo-hi halves
  - P@V lags scores by one window (software pipeline) feeding exp on ACT
    (scale=1/8 folded in, no max-subtraction); [V|1] stationary gives the
    denominator row for free; [V|1] ones columns are baked into the V
    projection (zero weight cols + bias 1)
  - po drains PSUM->SBUF immediately at each accumulation stop so the next
    group starts without waiting for normalization (K=1 broadcast matmul +
    reciprocal, emitted lazily inside the next phase's window loop)
  - odd-tile V projections and even-tile O-projection tiles are interleaved
    into stage B window loops to fill PE gaps; O-projection packs h0+h1 into
    one [128,128] stationary (true head-sum via 128-deep contraction)
"""

import sys

sys.path.insert(0, "/opt/trn_rl_repo")

import numpy as np
import ml_dtypes

BF16 = ml_dtypes.bfloat16

B = 2
S = 2048
D = 768
DK = 64
WIN = 256
NW = S // WIN   # 8 windows
NHC = 3         # heads per core
DH = NHC * DK   # 192
NT = S // 128   # 16 q tiles

_CACHE = {}


def _build_program():
    import concourse.tile as tile
    from concourse import mybir, bacc
    from contextlib import ExitStack

    f32 = mybir.dt.float32
    bf16 = mybir.dt.bfloat16
    Exp = mybir.ActivationFunctionType.Exp
    mult = mybir.AluOpType.mult
    add = mybir.AluOpType.add

    nc = bacc.Bacc("TRN2", target_bir_lowering=False, debug=False)

    xT = nc.dram_tensor("xT", [D, S], bf16, kind="ExternalInput").ap()
    w1 = nc.dram_tensor("w1", [D, 256], bf16, kind="ExternalInput").ap()  # [qh0|qh1|kh0|kh1]
    w2 = nc.dram_tensor("w2", [D, 256], bf16, kind="ExternalInput").ap()  # [qh2|qh2|kh2|kh2]
    wv = nc.dram_tensor("wv", [D, 195], bf16, kind="ExternalInput").ap()
    woA = nc.dram_tensor("woA", [128, D], bf16, kind="ExternalInput").ap()  # WoT h0|h1
    woB = nc.dram_tensor("woB", [64, D], bf16, kind="ExternalInput").ap()   # WoT h2
    btA = nc.dram_tensor("btA", [128, 1], f32, kind="ExternalInput").ap()
    btB = nc.dram_tensor("btB", [128, 1], f32, kind="ExternalInput").ap()
    btC = nc.dram_tensor("btC", [128, 1], f32, kind="ExternalInput").ap()
    btD = nc.dram_tensor("btD", [128, 1], f32, kind="ExternalInput").ap()
    bvb = nc.dram_tensor("bvb", [128, 195], bf16, kind="ExternalInput").ap()
    triu = nc.dram_tensor("triu", [128, 128], bf16, kind="ExternalInput").ap()
    onesd = nc.dram_tensor("onesd", [128, 64], bf16, kind="ExternalInput").ap()
    out = nc.dram_tensor("out", [S, D], bf16, kind="ExternalOutput").ap()

    with tile.TileContext(nc) as tc, ExitStack() as ctx:
        consts = ctx.enter_context(tc.tile_pool(name="consts", bufs=1))
        qkv = ctx.enter_context(tc.tile_pool(name="qkv", bufs=1))

        xtp_cm = tc.tile_pool(name="xtp", bufs=1)
        xtp = xtp_cm.__enter__()
        xT_sb = [xtp.tile([128, S], bf16, tag=f"xt{k}", name=f"xt{k}")
                 for k in range(6)]
        w1_sb = [consts.tile([128, 256], bf16, tag=f"w1_{k}", name=f"w1s{k}")
                 for k in range(6)]
        w2_sb = [consts.tile([128, 256], bf16, tag=f"w2_{k}", name=f"w2s{k}")
                 for k in range(6)]
        wv_sb = [consts.tile([128, 195], bf16, tag=f"wv_{k}", name=f"wvs{k}")
                 for k in range(6)]
        for k in range(6):
            nc.sync.dma_start(out=xT_sb[k], in_=xT[k * 128:(k + 1) * 128, :])
            nc.sync.dma_start(out=w1_sb[k], in_=w1[k * 128:(k + 1) * 128, :])
            nc.sync.dma_start(out=w2_sb[k], in_=w2[k * 128:(k + 1) * 128, :])
            nc.sync.dma_start(out=wv_sb[k], in_=wv[k * 128:(k + 1) * 128, :])
        woA_sb = consts.tile([128, D], bf16, tag="woA")
        woB_sb = consts.tile([64, D], bf16, tag="woB")
        nc.sync.dma_start(out=woA_sb, in_=woA)
        nc.sync.dma_start(out=woB_sb, in_=woB)
        btA_sb = consts.tile([128, 1], f32, tag="btA")
        btB_sb = consts.tile([128, 1], f32, tag="btB")
        btC_sb = consts.tile([128, 1], f32, tag="btC")
        btD_sb = consts.tile([128, 1], f32, tag="btD")
        nc.sync.dma_start(out=btA_sb, in_=btA)
        nc.sync.dma_start(out=btB_sb, in_=btB)
        nc.sync.dma_start(out=btC_sb, in_=btC)
        nc.sync.dma_start(out=btD_sb, in_=btD)
        bvb_sb = consts.tile([128, 195], bf16, tag="bvb")
        nc.sync.dma_start(out=bvb_sb, in_=bvb)
        triu_sb = consts.tile([128, 128], bf16, tag="triu")
        nc.sync.dma_start(out=triu_sb, in_=triu)
        ones_row = consts.tile([128, 64], bf16, tag="ones_row")
        nc.sync.dma_start(out=ones_row, in_=onesd)

        # ---- long-lived activation tiles ----
        tileA = qkv.tile([128, S], bf16, tag="tileA")  # [qT_h0|qT_h1], q-permuted
        tileB = qkv.tile([128, S], bf16, tag="tileB")  # [kT_h0|kT_h1], natural
        tileC = qkv.tile([128, S], bf16, tag="tileC")  # qT_h2 x2, permuted
        tileD = qkv.tile([128, S], bf16, tag="tileD")  # kT_h2 x2, natural
        # v natural [s,d] per s-tile: three 65-col groups [V_h | 1]
        v_sb = [qkv.tile([128, 196], bf16, tag=f"v{i}", name=f"vsb{i}")
                for i in range(NT)]
        # attn^T: h0 at partitions 0:64, h1 at 64:128; h2 separate
        attnT01 = qkv.tile([128, S], bf16, tag="attnT01")
        attnT2 = qkv.tile([64, S], bf16, tag="attnT2")

        def mm(out, lhsT, rhs, start, stop, reuse=False, **kw):
            return nc.tensor.matmul(out, lhsT, rhs, start=start, stop=stop,
                                    **kw)

        def permuted_copy(dst, rows, ps, n, bias):
            """psum 512-span n -> dst cols with even/odd tile permutation."""
            pr3 = ps[0:rows, :].rearrange("p (c two k) -> p c two k", two=2, k=128)
            dr = dst[0:rows, :]
            nc.vector.tensor_scalar_add(
                out=dr[:, 256 * n:256 * n + 256].rearrange("p (c k) -> p c k", k=128),
                in0=pr3[:, :, 0, :], scalar1=bias[0:rows, :])
            nc.vector.tensor_scalar_add(
                out=dr[:, 1024 + 256 * n:1024 + 256 * n + 256].rearrange(
                    "p (c k) -> p c k", k=128),
                in0=pr3[:, :, 1, :], scalar1=bias[0:rows, :])

        # ---- stage A ----
        with tc.tile_pool(name="psA", bufs=2, space="PSUM") as psA:
            for n in range(4):
                xn = [xT_sb[k][:, 512 * n:512 * (n + 1)]
                      for k in range(6)]
                psa = psA.tile([128, 512], f32, tag="psA")
                for k in range(6):
                    nc.tensor.matmul(psa, w1_sb[k][:, 0:128], xn[k],
                                     start=(k == 0), stop=(k == 5))
                permuted_copy(tileA, 128, psa, n, btA_sb)
                psb = psA.tile([128, 512], f32, tag="psA")
                for k in range(6):
                    nc.tensor.matmul(psb, w1_sb[k][:, 128:256], xn[k],
                                     start=(k == 0), stop=(k == 5))
                nc.vector.tensor_scalar_add(
                    out=tileB[:, 512 * n:512 * (n + 1)], in0=psb, scalar1=btB_sb)
                psq = psA.tile([128, 512], f32, tag="psq")
                psk = psA.tile([128, 512], f32, tag="psq")
                for k in range(6):
                    nc.tensor.matmul(psq, w2_sb[k][:, 0:128], xn[k],
                                     start=(k == 0), stop=(k == 5))
                    nc.tensor.matmul(psk, w2_sb[k][:, 128:256], xn[k],
                                     start=(k == 0), stop=(k == 5))
                permuted_copy(tileC, 128, psq, n, btC_sb)
                nc.vector.tensor_scalar_add(
                    out=tileD[:, 512 * n:512 * (n + 1)], in0=psk, scalar1=btD_sb)

            def v_proj(st, psv):
                for k in range(6):
                    nc.tensor.matmul(
                        psv, xT_sb[k][:, 128 * st:128 * (st + 1)],
                        wv_sb[k], start=(k == 0), stop=(k == 5))
                vt = v_sb[st]
                # ones columns baked in: wv cols {64,129,194}=0, bvb there=1
                nc.vector.tensor_tensor(
                    out=vt[:, 0:195], in0=psv, in1=bvb_sb, op=add)

            # even s-tiles now (feed grp0's P@V); odd tiles deferred into
            # stage B's grp0 loop to fill PE gaps there
            for st in range(0, NT, 2):
                psv = psA.tile([128, 195], f32, tag="psv")
                v_proj(st, psv)

        # ---- stage B + interleaved stage C ----
        triu_b = triu_sb.unsqueeze(1).broadcast_to([128, 8, 128])

        with tc.tile_pool(name="pt", bufs=8) as pt_pool, \
             tc.tile_pool(name="sc", bufs=2, space="PSUM") as sc_pool, \
             tc.tile_pool(name="po", bufs=2, space="PSUM") as out_pool, \
             tc.tile_pool(name="posb", bufs=2) as posb_pool, \
             tc.tile_pool(name="nrm", bufs=2) as nrm_pool, \
             tc.tile_pool(name="ost", bufs=3) as ost_pool:

            def scores_exp(kblk, qcols, mask, nm):
                sc = sc_pool.tile([128, 1024], f32, tag="sc")
                for sub in range(2):
                    mm(sc[:, 512 * sub:512 * (sub + 1)], kblk,
                       qcols[:, 512 * sub:512 * (sub + 1)],
                       start=True, stop=True, reuse=(sub == 1))
                pt = pt_pool.tile([128, 1024], bf16, tag="pt", name=nm)
                nc.scalar.activation(out=pt, in_=sc, func=Exp, scale=0.125)
                if mask:
                    p3 = pt.rearrange("p (c k) -> p c k", k=128)
                    nc.vector.tensor_mul(out=p3, in0=p3, in1=triu_b)
                return pt

            def scores_exp_pair(kblks, qcolss, masks, nm):
                """two K=64 score blocks on disjoint row-groups (partitions
                0:64 / 64:128) with the matmuls interleaved so adjacent MMs
                can run concurrently as row tiles in the PE array."""
                scs = [sc_pool.tile([128, 1024], f32, tag="sc",
                                    name=f"{nm}s{h}") for h in range(2)]
                for sub in range(2):
                    for h in range(2):
                        mm(scs[h][:, 512 * sub:512 * (sub + 1)], kblks[h],
                           qcolss[h][:, 512 * sub:512 * (sub + 1)],
                           start=True, stop=True, reuse=(sub == 1))
                pts = []
                for h in range(2):
                    pt = pt_pool.tile([128, 1024], bf16, tag="pt",
                                      name=f"{nm}p{h}")
                    nc.scalar.activation(out=pt, in_=scs[h], func=Exp,
                                         scale=0.125)
                    pts.append(pt)
                for h in range(2):
                    if masks[h]:
                        p3 = pts[h].rearrange("p (c k) -> p c k", k=128)
                        nc.vector.tensor_mul(out=p3, in0=p3, in1=triu_b)
                return pts

            def pv_mm(po, vtile, h, pt, first, last):
                vsl = vtile[:, 65 * h:65 * h + 65]  # [V_h | 1]
                for sub in range(2):
                    mm(po[0:65, 512 * sub:512 * (sub + 1)], vsl,
                       pt[:, 512 * sub:512 * (sub + 1)],
                       start=first, stop=last, reuse=(sub == 1))

            def drain(po, nm):
                """po [65,1024] psum -> SBUF bf16, freeing the psum bank."""
                po_sb = posb_pool.tile([65, 1024], bf16, tag="posb", name=nm)
                nc.vector.tensor_copy(out=po_sb, in_=po[0:65, :])
                return po_sb

            def norm_emit(po_sb, dstv, grp):
                """normalize drained po into an attnT slice (PE+DVE, lazy)."""
                rec_ps = sc_pool.tile([128, 1024], f32, tag="sc")
                for sub in range(2):
                    mm(rec_ps[0:64, 512 * sub:512 * (sub + 1)],
                       ones_row[64:65, :],
                       po_sb[64:65, 512 * sub:512 * (sub + 1)],
                       start=True, stop=True, reuse=(sub == 1))
                rec_sb = nrm_pool.tile([64, 1024], f32, tag="rec")
                nc.vector.reciprocal_approx_fast(
                    out=rec_sb, in_=rec_ps[0:64, :])
                nc.vector.tensor_tensor(
                    out=dstv[:, 1024 * grp:1024 * (grp + 1)],
                    in0=po_sb[0:64, :], in1=rec_sb, op=mult)

            def c_tile(p, copy_eng, pool=None, ptag="sc"):
                """one O-projection tile: 4 MMs + psum->sbuf copy + DMA."""
                psof = (pool or sc_pool).tile([128, 1024], f32, tag=ptag,
                                              name=f"pso{p}")
                pso = psof[:, 0:D]
                s01 = attnT01[:, 128 * p:128 * (p + 1)]
                s2 = attnT2[:, 128 * p:128 * (p + 1)]
                for i, (n0, n1) in enumerate(((0, 512), (512, 768))):
                    mm(pso[:, n0:n1], s01, woA_sb[:, n0:n1],
                       start=True, stop=False, reuse=(i == 1))
                for i, (n0, n1) in enumerate(((0, 512), (512, 768))):
                    mm(pso[:, n0:n1], s2, woB_sb[:, n0:n1],
                       start=False, stop=True, reuse=(i == 1))
                ot = ost_pool.tile([128, D], bf16, tag="ot")
                copy_eng(out=ot, in_=pso)
                t = 2 * p if p < 8 else 2 * (p - 8) + 1
                nc.sync.dma_start(out=out[128 * t:128 * (t + 1), :], in_=ot)

            # --- h0 & h1 row-tile-paired; PV lags scores by one window;
            #     odd-tile V projections fill grp0's PE gaps ---
            pending_norm = []
            for grp in range(2):  # 0=even q-tiles, 1=odd
                po01 = [out_pool.tile([128, 1024], f32, tag="po",
                                      name=f"po{grp}_{i}")
                        for i in range(2)]
                qvs = [tileA[64 * h:64 * h + 64,
                             1024 * grp:1024 * (grp + 1)] for h in range(2)]
                pend = [None, None]  # per head: list of (pt, vtile, first)
                for w in range(NW):
                    klos = [tileB[64 * h:64 * h + 64, WIN * w:WIN * w + 128]
                            for h in range(2)]
                    if grp == 0:
                        pts = scores_exp_pair(klos, qvs, (True, True), f"g0w{w}")
                        blocks = [[(pts[h], v_sb[2 * w])] for h in range(2)]
                    else:
                        ptl = scores_exp_pair(klos, qvs, (False, False), f"g1w{w}l")
                        khis = [tileB[64 * h:64 * h + 64,
                                      WIN * w + 128:WIN * w + 256]
                                for h in range(2)]
                        pth = scores_exp_pair(khis, qvs, (True, True), f"g1w{w}u")
                        blocks = [[(ptl[h], v_sb[2 * w]),
                                   (pth[h], v_sb[2 * w + 1])]
                                  for h in range(2)]
                    for h in range(2):
                        if pend[h] is not None:
                            for pt_, vt_, fi_ in pend[h]:
                                pv_mm(po01[h], vt_, h, pt_, fi_, False)
                        pend[h] = [(b[0], b[1], (w == 0 and i == 0))
                                   for i, b in enumerate(blocks[h])]
                    if grp == 0:
                        # odd V tile 2w+1 (psum borrowed from the sc pool)
                        psvf = sc_pool.tile([128, 1024], f32, tag="sc",
                                            name=f"psv{w}")
                        v_proj(2 * w + 1, psvf[:, 0:195])
                    if w == 1 and pending_norm:
                        for args in pending_norm:
                            norm_emit(*args)
                        pending_norm = []
                for h in range(2):
                    n = len(pend[h])
                    for i, (pt_, vt_, fi_) in enumerate(pend[h]):
                        pv_mm(po01[h], vt_, h, pt_, fi_, i == n - 1)
                    po_sb = drain(po01[h], f"posb{grp}_{h}")
                    pending_norm.append(
                        (po_sb, attnT01[64 * h:64 * h + 64, :], grp))


            # --- h2 (q/k duplicated on both partition halves so blocks
            #     row-tile-pair); stage C even q-tiles interleaved in grp1 ---
            for grp in range(2):
                po = out_pool.tile([128, 1024], f32, tag="po",
                                   name=f"po2_{grp}")
                qvh = [tileC[64 * j:64 * j + 64,
                             1024 * grp:1024 * (grp + 1)] for j in range(2)]
                pend = None
                for it in range(4 if grp == 0 else NW):
                    if grp == 0:
                        # windows 2it (rows 0:64) and 2it+1 (rows 64:128)
                        w0, w1 = 2 * it, 2 * it + 1
                        kbs = [tileD[0:64, WIN * w0:WIN * w0 + 128],
                               tileD[64:128, WIN * w1:WIN * w1 + 128]]
                        pts = scores_exp_pair(kbs, qvh, (True, True),
                                              f"h2g0i{it}")
                        blocks = [(pts[0], v_sb[2 * w0]),
                                  (pts[1], v_sb[2 * w1])]
                    else:
                        w = it
                        kbs = [tileD[0:64, WIN * w:WIN * w + 128],
                               tileD[64:128, WIN * w + 128:WIN * w + 256]]
                        pts = scores_exp_pair(kbs, qvh, (False, True),
                                              f"h2g1i{it}")
                        blocks = [(pts[0], v_sb[2 * w]),
                                  (pts[1], v_sb[2 * w + 1])]
                    if pend is not None:
                        for pt_, vt_, fi_ in pend:
                            pv_mm(po, vt_, 2, pt_, fi_, False)
                    pend = [(b[0], b[1], (it == 0 and i == 0))
                            for i, b in enumerate(blocks)]
                    if it == 1 and pending_norm:
                        for args in pending_norm:
                            norm_emit(*args)
                        pending_norm = []
                    if grp == 1 and it >= 1:
                        c_tile(it - 1, nc.vector.tensor_copy)
                n = len(pend)
                for i, (pt_, vt_, fi_) in enumerate(pend):
                    pv_mm(po, vt_, 2, pt_, fi_, i == n - 1)
                po_sb = drain(po, f"posb2_{grp}")
                pending_norm.append((po_sb, attnT2, grp))

            c_tile(7, nc.vector.tensor_copy)
            for args in pending_norm:
                norm_emit(*args)
            pending_norm = []

            # --- stage C tail: odd q-tiles, two psum pools in flight ---
            engs = [nc.scalar.copy, nc.vector.tensor_copy]
            for p in range(8, NT):
                if p % 2:
                    c_tile(p, engs[p % 2], pool=out_pool, ptag="po")
                else:
                    c_tile(p, engs[p % 2])

        xtp_cm.__exit__(None, None, None)

    nc.compile()
    return nc


def _prep_core_inputs(inputs, c):
    x = inputs["x"]
    Wq, bq = inputs["Wq"], inputs["bq"]
    Wk, bk = inputs["Wk"], inputs["bk"]
    Wv, bv = inputs["Wv"], inputs["bv"]
    Wo = inputs["Wo"]
    b = c // 4
    r0 = (c % 4) * DH  # first feature row of this core's 192-row head block

    xT = np.ascontiguousarray(np.asarray(x[b]).T.astype(BF16))
    W1 = np.ascontiguousarray(np.concatenate(
        [Wq[r0:r0 + 128].T, Wk[r0:r0 + 128].T], axis=1).astype(BF16))
    q2 = Wq[r0 + 128:r0 + 192].T
    k2 = Wk[r0 + 128:r0 + 192].T
    W2 = np.ascontiguousarray(np.concatenate([q2, q2, k2, k2], axis=1).astype(BF16))
    Wvp = np.zeros((D, 195), np.float32)
    for h in range(3):
        Wvp[:, 65 * h:65 * h + 64] = Wv[r0 + 64 * h:r0 + 64 * h + 64].T
    Wvp = np.ascontiguousarray(Wvp.astype(BF16))
    bvbr = np.zeros((195,), np.float32)
    for h in range(3):
        bvbr[65 * h:65 * h + 64] = bv[r0 + 64 * h:r0 + 64 * h + 64]
        bvbr[65 * h + 64] = 1.0
    woA = np.ascontiguousarray(Wo[:, r0:r0 + 128].T.astype(BF16))
    woB = np.ascontiguousarray(Wo[:, r0 + 128:r0 + 192].T.astype(BF16))

    return dict(
        xT=xT, w1=W1, w2=W2, wv=Wvp, woA=woA, woB=woB,
        btA=np.ascontiguousarray(bq[r0:r0 + 128].reshape(128, 1).astype(np.float32)),
        btB=np.ascontiguousarray(bk[r0:r0 + 128].reshape(128, 1).astype(np.float32)),
        btC=np.ascontiguousarray(np.tile(
            bq[r0 + 128:r0 + 192], 2).reshape(128, 1).astype(np.float32)),
        btD=np.ascontiguousarray(np.tile(
            bk[r0 + 128:r0 + 192], 2).reshape(128, 1).astype(np.float32)),
        bvb=np.ascontiguousarray(np.tile(
            bvbr.reshape(1, 195), (128, 1)).astype(BF16)),
        triu=np.ascontiguousarray(np.triu(np.ones((128, 128), np.float32))).astype(BF16),
        onesd=np.ones((128, 64), BF16),
    )


def _install_ntff_hook():
    """Register antenv.axon_hooks with a ctypes NTFF profile hook so
    run_bass_kernel_spmd(trace=True) can capture device-side exec time."""
    import types, ctypes, contextlib

    try:
        import antenv.axon_hooks  # noqa: F401
        return
    except ImportError:
        pass
    so_path = "/opt/axon/libaxon_pjrt.so"
    lib = ctypes.CDLL(so_path)
    if not hasattr(lib, "axon_start_nrt_profile"):
        return
    lib.axon_start_nrt_profile.argtypes = [
        ctypes.POINTER(ctypes.c_int64), ctypes.c_size_t]
    lib.axon_start_nrt_profile.restype = ctypes.c_int64
    lib.axon_stop_nrt_profile.argtypes = [ctypes.c_char_p]
    lib.axon_stop_nrt_profile.restype = ctypes.c_int64

    @contextlib.contextmanager
    def _hook(output_dir, device_ids):
        import jax
        jax.devices()
        if device_ids:
            ids = (ctypes.c_int64 * len(device_ids))(*device_ids)
            rc = lib.axon_start_nrt_profile(ids, len(device_ids))
        else:
            rc = lib.axon_start_nrt_profile(None, 0)
        if rc != 0:
            raise RuntimeError(f"axon_start_nrt_profile rc={rc}")
        try:
            yield
        finally:
            n = lib.axon_stop_nrt_profile(str(output_dir).encode())
            print(f"profile: {n} file(s) written to {output_dir}")

    mod = types.ModuleType("antenv.axon_hooks")
    mod.get_axon_ntff_profile_hook = lambda: _hook
    mod.set_axon_ntff_profile_hook = lambda h: None
    sys.modules["antenv.axon_hooks"] = mod
    import antenv
    antenv.axon_hooks = mod


def kernel(**inputs):
    import os
    from concourse import bass_utils

    if "nc" not in _CACHE:
        _CACHE["nc"] = _build_program()
    nc = _CACHE["nc"]

    trace = bool(os.environ.get("MHA_TRACE"))
    kwargs = {}
    if trace:
        _install_ntff_hook()
        kwargs = dict(trace=True, tmpdir="/tmp/mha_trace")
        os.makedirs("/tmp/mha_trace", exist_ok=True)

    in_maps = [_prep_core_inputs(inputs, c) for c in range(8)]
    res = bass_utils.run_bass_kernel_spmd(
        nc, in_maps, core_ids=list(range(8)), **kwargs)
    _CACHE["last_results"] = res
    if trace and res.exec_time_ns is not None:
        print(f"HW exec time: {res.exec_time_ns} ns")
    out = np.zeros((B, S, D), np.float32)
    for c in range(8):
        out[c // 4] += res.results[c]["out"].astype(np.float32)
    out += np.asarray(inputs["bo"], np.float32).reshape(1, 1, D)
    return out
